# revision 24
# baseline (speedup 1.0000x reference)
"""Trainium2 Bass kernel for the GCN model (nn_GCNModel_57853209477141).

Model: 3x GCNConv(128->128, sym-norm with self loops) with ReLU, question
embedding MLP, concat, 2-layer MLP head -> [50000, 32].

v2 design (8 NeuronCores, single SPMD launch):
- dst-node sharding: tiles of 128 nodes snake-dealt to cores by edge count;
  one compile-time schedule serves all 8 cores (SPMD).
- GCN norm factorization: agg[v] = dinv[v] * sum_e (dinv*h)[src_e]; tables
  store h~ = dinv*h in bf16, per-edge norm disappears.
- gather primitive: gpsimd.dma_gather (InstDMAGatherAnt), <=8 chunks
  (1024 rows) per instruction, int16 indices wrapped over 16 partitions.
  Table split at row 32768 into lo/hi halves to fit int16 range; each
  slot's edge stream is [lo chunks..., hi chunks...].
- aggregation per chunk: 0/1 one-hot [edge,dst] built on DVE via
  iota-compare, matmul-accumulated (bf16) into PSUM; self-loop term added
  from the resident h~ slice via identity matmul.
- single block-major table layout for all 3 layers; AllGather split into
  7 row-blocks fired as production completes (incl. layer 0) so collective
  traffic overlaps compute/gather.
- question path computed on host (0.1% of FLOPs), expanded per node with
  fc1 bias folded, loaded as a constant.

Host preprocessing: index/layout work + the tiny question MLP; all O(E*F)
and O(N*F*F) float work runs on device.
"""
import os
import sys
import types
from contextlib import ExitStack

import numpy as np

# ---------------------------------------------------------------- constants
N = 50000
E = 800000
G = 64
P = 128
NCORES = 8
TPC = 49  # tile slots per core
SPB = 7   # slots per AllGather block
NBLK = TPC // SPB  # 7
SLOT_ROWS = TPC * P  # 6272
NT = NCORES * SLOT_ROWS  # 50176
QD = 768
OUTC = 32
HALF = 32768  # int16 index split point
MAXC = 8  # max chunks (1024 rows) per dma_gather piece


def _install_axon_prof():
    """Register NTFF profile hook if the image's antenv lacks it; neuter
    bucket upload (zero-egress). Harmless when running without tracing."""
    try:
        from antenv import axon_hooks  # noqa: F401
    except ImportError:
        try:
            import antenv
            from trn_agent_boot.trn_boot import _ntff_profile_via_ctypes

            hook = _ntff_profile_via_ctypes("/opt/axon/libaxon_pjrt.so")
            mod = types.ModuleType("antenv.axon_hooks")
            mod.get_axon_ntff_profile_hook = lambda: hook
            mod.set_axon_ntff_profile_hook = lambda h: None
            sys.modules["antenv.axon_hooks"] = mod
            antenv.axon_hooks = mod
        except Exception:
            pass
    try:
        import concourse.bass_utils as bu

        bu.upload_artifacts = lambda tmpdir: "local://" + str(tmpdir)
    except Exception:
        pass


def _wrap_idxs16(lin: np.ndarray) -> np.ndarray:
    """[n] int -> [128, n//16] int16: wrapped col-major over 16 partitions
    (element i -> [i%16, i//16]), replicated 8x across partition groups."""
    n = lin.shape[0]
    assert n % 16 == 0
    w = np.asarray(lin, dtype=np.int16).reshape(n // 16, 16).T
    return np.tile(w, (8, 1))


# ---------------------------------------------------------------- host prep
def preprocess(edge_index, batch):
    src = np.asarray(edge_index[0], dtype=np.int64)
    dst = np.asarray(edge_index[1], dtype=np.int64)
    deg = (np.bincount(dst, minlength=N) + 1).astype(np.float64)
    dinv = np.where(deg > 0, 1.0 / np.sqrt(deg), 0.0).astype(np.float32)

    n_tiles = (N + P - 1) // P  # 391
    tile_of_node = np.arange(N) // P
    dst_tile = dst // P
    tile_counts = np.bincount(dst_tile, minlength=n_tiles)

    # snake-deal tiles (sorted by edge count desc) across cores
    order_all = np.argsort(-tile_counts, kind="stable")
    core_tiles = [[] for _ in range(NCORES)]
    for r in range(TPC):
        batch_t = order_all[r * NCORES : (r + 1) * NCORES]
        seq = range(NCORES) if r % 2 == 0 else range(NCORES - 1, -1, -1)
        for j, c in enumerate(seq):
            core_tiles[c].append(int(batch_t[j]) if j < len(batch_t) else -1)
    core_of_tile = np.full(n_tiles, -1, dtype=np.int64)
    slot_of_tile = np.full(n_tiles, -1, dtype=np.int64)
    for c in range(NCORES):
        for s, t in enumerate(core_tiles[c]):
            if t >= 0:
                core_of_tile[t] = c
                slot_of_tile[t] = s

    # block-major table row: [block][core][slot%SPB][pos]
    blk = slot_of_tile[tile_of_node] // SPB
    table_row = (
        blk * (NCORES * SPB * P)
        + core_of_tile[tile_of_node] * (SPB * P)
        + (slot_of_tile[tile_of_node] % SPB) * P
        + (np.arange(N) % P)
    )

    order = np.argsort(dst_tile, kind="stable")
    src_sorted = src[order]
    dst_sorted = dst[order]
    sorted_tiles = dst_tile[order]
    tile_starts = np.searchsorted(sorted_tiles, np.arange(n_tiles))
    tile_ends = np.searchsorted(sorted_tiles, np.arange(n_tiles), side="right")

    # per (core, slot): lo/hi edge streams (by table_row of src)
    lo_idx = [[None] * TPC for _ in range(NCORES)]
    hi_idx = [[None] * TPC for _ in range(NCORES)]
    lo_dst = [[None] * TPC for _ in range(NCORES)]
    hi_dst = [[None] * TPC for _ in range(NCORES)]
    for c in range(NCORES):
        for s in range(TPC):
            t = core_tiles[c][s]
            if t < 0:
                lo_idx[c][s] = np.zeros(0, np.int64)
                hi_idx[c][s] = np.zeros(0, np.int64)
                lo_dst[c][s] = np.zeros(0, np.int64)
                hi_dst[c][s] = np.zeros(0, np.int64)
                continue
            a, b = tile_starts[t], tile_ends[t]
            rows = table_row[src_sorted[a:b]]
            din = dst_sorted[a:b] % P
            m = rows < HALF
            # sort lo edges by table row: early gather pieces then only
            # depend on the first AllGather blocks (tighter base bound)
            rl, dl = rows[m], din[m]
            o = np.argsort(rl, kind="stable")
            lo_idx[c][s] = rl[o]
            lo_dst[c][s] = dl[o]
            hi_idx[c][s] = rows[~m] - HALF
            hi_dst[c][s] = din[~m]

    chunks_lo = np.zeros(TPC, dtype=np.int64)
    chunks_hi = np.zeros(TPC, dtype=np.int64)
    for s in range(TPC):
        for c in range(NCORES):
            chunks_lo[s] = max(chunks_lo[s], (len(lo_idx[c][s]) + P - 1) // P)
            chunks_hi[s] = max(chunks_hi[s], (len(hi_idx[c][s]) + P - 1) // P)
    nch = chunks_lo + chunks_hi
    TCH = int(nch.sum())
    TCHL = int(chunks_lo.sum())
    TCHH = int(chunks_hi.sum())
    lo_base = np.cumsum(np.concatenate([[0], chunks_lo[:-1]])).astype(int)
    hi_base = np.cumsum(np.concatenate([[0], chunks_hi[:-1]])).astype(int)

    # per-layer lo/hi chunk streams (slot-major); gather pieces of MAXC
    # chunks cut across slot boundaries so nearly all pieces are full.
    # dstin columns: lo stream at [0, TCHL), hi stream at [TCHL, TCH).
    dstin_T = np.full((NCORES, P, TCH), -1.0, dtype=np.float32)
    idxW = np.zeros((NCORES, P, 8 * TCH), dtype=np.int16)
    piece0_max = []
    for c in range(NCORES):
        lo_lin = np.zeros(TCHL * P, dtype=np.int64)
        lo_dl = np.full(TCHL * P, -1.0, dtype=np.float32)
        hi_lin = np.zeros(TCHH * P, dtype=np.int64)
        hi_dl = np.full(TCHH * P, -1.0, dtype=np.float32)
        for s in range(TPC):
            o = lo_base[s] * P
            nl = len(lo_idx[c][s])
            lo_lin[o : o + nl] = lo_idx[c][s]
            lo_dl[o : o + nl] = lo_dst[c][s]
            o = hi_base[s] * P
            nh = len(hi_idx[c][s])
            hi_lin[o : o + nh] = hi_idx[c][s]
            hi_dl[o : o + nh] = hi_dst[c][s]
        dstin_T[c, :, :TCHL] = lo_dl.reshape(TCHL, P).T
        dstin_T[c, :, TCHL:] = hi_dl.reshape(TCHH, P).T
        if TCHL:
            idxW[c, :, : 8 * TCHL] = _wrap_idxs16(lo_lin)
        if TCHH:
            idxW[c, :, 8 * TCHL :] = _wrap_idxs16(hi_lin)
        piece0_max.append(int(lo_lin[: MAXC * P].max()) if TCHL else 0)

    dinv_slot = np.zeros((NCORES, P, TPC), dtype=np.float32)
    gid_slot = np.zeros((NCORES, P, TPC), dtype=np.int64)
    node_perm = np.full((NCORES, SLOT_ROWS), -1, dtype=np.int64)
    batch = np.asarray(batch, dtype=np.int64)
    for c in range(NCORES):
        for s in range(TPC):
            t = core_tiles[c][s]
            if t < 0:
                continue
            v0 = t * P
            v1 = min(v0 + P, N)
            n = v1 - v0
            dinv_slot[c, :n, s] = dinv[v0:v1]
            gid_slot[c, :n, s] = batch[v0:v1]
            node_perm[c, s * P : s * P + n] = np.arange(v0, v1)

    # piece-0 base bound, quantized up to AG-block rows (block = 7168 rows)
    blk_rows = NCORES * SPB * P
    b0 = max(piece0_max) + 1 if piece0_max else HALF
    piece0_bound = min(-(-b0 // blk_rows) * blk_rows, NT)

    return dict(
        chunks_lo=chunks_lo,
        chunks_hi=chunks_hi,
        piece0_bound=piece0_bound,
        TCH=TCH,
        TCHL=TCHL,
        TCHH=TCHH,
        lo_base=lo_base,
        hi_base=hi_base,
        dstin_T=dstin_T,
        idxW=idxW,
        dinv_slot=dinv_slot,
        gid_slot=gid_slot,
        node_perm=node_perm,
    )


# ------------------------------------------------------------- bass program
def build_program(schedule):
    import concourse.bacc as bacc
    import concourse.bass as bass
    import concourse.tile as tile
    from concourse import mybir
    from concourse.masks import make_identity

    F32 = mybir.dt.float32
    BF16 = mybir.dt.bfloat16
    I16 = mybir.dt.int16
    chunks_lo = schedule["chunks_lo"]
    chunks_hi = schedule["chunks_hi"]
    piece0_bound = schedule["piece0_bound"]
    TCH = schedule["TCH"]
    TCHL = schedule["TCHL"]
    TCHH = schedule["TCHH"]
    lo_base = schedule["lo_base"]
    hi_base = schedule["hi_base"]

    nc = bacc.Bacc(
        "TRN2", target_bir_lowering=False, dynamic_dma_scratch_size=32768
    )
    dp = nc.declare_dram_parameter
    xT = dp("xT", [P, SLOT_ROWS], BF16, isOutput=False)
    idxW_in = dp("idxW", [P, 8 * TCH], I16, isOutput=False)
    dstin = dp("dstin", [P, TCH], BF16, isOutput=False)
    iota_in = dp("iota_in", [P, P], BF16, isOutput=False)
    dinv_in = dp("dinv_in", [P, TPC], F32, isOutput=False)
    W0_in = dp("W0", [P, P], BF16, isOutput=False)
    W1_in = dp("W1", [P, P], F32, isOutput=False)
    W2_in = dp("W2", [P, P], F32, isOutput=False)
    bb_in = [dp(f"bb{i}", [P, P], F32, isOutput=False) for i in range(3)]
    fc1a_in = dp("fc1a", [P, P], F32, isOutput=False)
    fc2w_in = dp("fc2w", [P, OUTC], F32, isOutput=False)
    fc2bb_in = dp("fc2bb", [P, OUTC], F32, isOutput=False)
    qgT_in = dp("qgT", [P, SLOT_ROWS], F32, isOutput=False)
    out_d = dp("out", [SLOT_ROWS, OUTC], F32, isOutput=True)

    cc_in = nc.dram_tensor("cc_in", [SLOT_ROWS, P], BF16)
    tables = [
        nc.dram_tensor(f"table{l}", [NT, P], BF16, addr_space="Shared")
        for l in range(3)
    ]

    # stream descriptors: (dstin col offset, idx col offset, total chunks)
    streams = [(0, 0, TCHL), (TCHL, 8 * TCHL, TCHH)]

    with tile.TileContext(nc) as tc, ExitStack() as ctx:
        const = ctx.enter_context(tc.tile_pool(name="const", bufs=1))
        gp = ctx.enter_context(tc.tile_pool(name="gp", bufs=9))
        ohp = ctx.enter_context(tc.tile_pool(name="ohp", bufs=8))
        psagg = ctx.enter_context(tc.tile_pool(name="psagg", bufs=3, space="PSUM"))
        psp = ctx.enter_context(tc.tile_pool(name="psp", bufs=3, space="PSUM"))
        pst = ctx.enter_context(tc.tile_pool(name="pst", bufs=2, space="PSUM"))
        epi = ctx.enter_context(tc.tile_pool(name="epi", bufs=3))

        # ---- constants (W0/dinv then xT in AG-block slices: production of
        # block 0 starts after ~230 KB instead of the whole 1.6 MB load)
        W0_sb = const.tile([P, P], BF16)
        nc.sync.dma_start(out=W0_sb[:], in_=W0_in[:])
        dinv_sb = const.tile([P, TPC], F32)
        nc.sync.dma_start(out=dinv_sb[:], in_=dinv_in[:])
        xT_sb = const.tile([P, SLOT_ROWS], BF16)
        for j in range(NBLK):
            c0, c1 = j * SPB * P, (j + 1) * SPB * P
            nc.sync.dma_start(out=xT_sb[:, c0:c1], in_=xT[:, c0:c1])
        idxW_sb = const.tile([P, 8 * TCH], I16)
        nc.scalar.dma_start(out=idxW_sb[:], in_=idxW_in[:])
        dstin_sb = const.tile([P, TCH], BF16)
        nc.scalar.dma_start(out=dstin_sb[:], in_=dstin[:])
        iota_sb = const.tile([P, P], BF16)
        nc.sync.dma_start(out=iota_sb[:], in_=iota_in[:])
        W_sb = [None] * 3
        for i, win in ((1, W1_in), (2, W2_in)):
            w = const.tile([P, P], F32, tag=f"W{i}")
            nc.sync.dma_start(out=w[:], in_=win[:])
            W_sb[i] = w
        bb_sb = []
        for i in range(3):
            b = const.tile([P, P], F32, tag=f"bb{i}")
            nc.sync.dma_start(out=b[:], in_=bb_in[i][:])
            bb_sb.append(b)
        fc1a_sb = const.tile([P, P], F32)
        nc.sync.dma_start(out=fc1a_sb[:], in_=fc1a_in[:])
        fc2w_sb = const.tile([P, OUTC], F32)
        nc.sync.dma_start(out=fc2w_sb[:], in_=fc2w_in[:])
        fc2bb_sb = const.tile([P, OUTC], F32)
        nc.sync.dma_start(out=fc2bb_sb[:], in_=fc2bb_in[:])
        qgT_sb = const.tile([P, SLOT_ROWS], F32)
        nc.scalar.dma_start(out=qgT_sb[:], in_=qgT_in[:])
        ident = const.tile([P, P], F32)
        make_identity(nc, ident[:])
        ident_r = const.tile([P, P], BF16, tag="ident_r")
        nc.vector.tensor_copy(out=ident_r[:], in_=ident[:])

        # resident own-slice h~ buffers (self-loop source), layer parity
        hs_keep = [
            const.tile([P, SLOT_ROWS], BF16, tag=f"hsk{i}", name=f"hsk{i}")
            for i in range(2)
        ]

        def allgather_block(l, j):
            r0 = j * SPB * P
            r1 = (j + 1) * SPB * P
            nc.gpsimd.collective_compute(
                "AllGather",
                mybir.AluOpType.bypass,
                replica_groups=[list(range(NCORES))],
                ins=[cc_in[r0:r1].opt()],
                outs=[
                    tables[l][
                        j * NCORES * SPB * P : (j + 1) * NCORES * SPB * P
                    ].opt()
                ],
            )

        # ---- layer 0 production: h~0 = dinv * (x @ W0), AG fired per block
        for s in range(TPC):
            pp = psp.tile([P, P], F32, space="PSUM", tag="mm")
            nc.tensor.matmul(
                out=pp[:],
                lhsT=xT_sb[:, s * P : (s + 1) * P],
                rhs=W0_sb[:],
                start=True,
                stop=True,
            )
            hs = hs_keep[0][:, s * P : (s + 1) * P]
            nc.scalar.activation(
                out=hs,
                in_=pp[:],
                func=mybir.ActivationFunctionType.Copy,
                scale=dinv_sb[:, s : s + 1],
            )
            nc.sync.dma_start(out=cc_in[s * P : (s + 1) * P, :], in_=hs)
            if (s + 1) % SPB == 0:
                allgather_block(0, s // SPB)

        # lo indices are < HALF, i.e. within AG blocks 0-4; bounding the
        # base AP there lets lo gathers start before AG blocks 5-6 land.
        LO_BOUND = 5 * NCORES * SPB * P  # 35840 >= HALF

        # ---- 3 aggregation layers
        for l in range(3):
            table = tables[l]
            piece_tiles = [{}, {}]  # per stream: piece id -> gather tile
            n_lo_pieces = (TCHL + MAXC - 1) // MAXC
            lo_issued = [0]

            def get_piece(h, pj):
                if pj in piece_tiles[h]:
                    return piece_tiles[h][pj]
                _, coloff, tot = streams[h]
                pc = min(MAXC, tot - MAXC * pj)
                g = gp.tile([P, MAXC * P], BF16, tag="g")
                if h == 0:
                    lb = piece0_bound if pj == 0 else LO_BOUND
                    base = table[:lb, :]
                else:
                    base = table[HALF:, :]
                colbase = coloff + 8 * MAXC * pj
                nc.gpsimd.dma_gather(
                    g[:, : pc * P].rearrange("p (k c) -> p k c", k=pc),
                    base,
                    idxW_sb[:, colbase : colbase + pc * 8],
                    pc * P,
                    pc * P,
                    P,
                )
                piece_tiles[h][pj] = g
                return g

            def prefetch_lo(upto):
                while lo_issued[0] < min(upto, n_lo_pieces):
                    get_piece(0, lo_issued[0])
                    lo_issued[0] += 1

            for s in range(TPC):
                prefetch_lo(int(lo_base[s]) // MAXC + 4)
                ps = psagg.tile([P, P], F32, space="PSUM", tag="agg")
                first = True
                for h, b0, cnt in (
                    (0, int(lo_base[s]), int(chunks_lo[s])),
                    (1, int(hi_base[s]), int(chunks_hi[s])),
                ):
                    doff = streams[h][0]
                    for k in range(cnt):
                        kk = b0 + k
                        g = get_piece(h, kk // MAXC)
                        kp = kk % MAXC
                        oh = ohp.tile([P, P], BF16, tag="oh")
                        nc.vector.tensor_tensor(
                            out=oh[:],
                            in0=dstin_sb[
                                :, doff + kk : doff + kk + 1
                            ].to_broadcast([P, P]),
                            in1=iota_sb[:],
                            op=mybir.AluOpType.is_equal,
                        )
                        nc.tensor.matmul(
                            out=ps[:],
                            lhsT=oh[:],
                            rhs=g[:, kp * P : (kp + 1) * P],
                            start=first,
                            stop=False,
                        )
                        first = False
                # self-loop term
                nc.tensor.matmul(
                    out=ps[:],
                    lhsT=ident_r[:],
                    rhs=hs_keep[l % 2][:, s * P : (s + 1) * P],
                    start=first,
                    stop=True,
                )
                # epilogue: h = relu(dinv*agg + b)
                t1 = epi.tile([P, P], F32, tag="t1")
                nc.scalar.activation(
                    out=t1[:],
                    in_=ps[:],
                    func=mybir.ActivationFunctionType.Copy,
                    scale=dinv_sb[:, s : s + 1],
                )
                t2 = epi.tile([P, P], F32, tag="t2")
                nc.vector.tensor_tensor(
                    out=t2[:], in0=t1[:], in1=bb_sb[l][:], op=mybir.AluOpType.add
                )
                hrelu = epi.tile([P, P], F32, tag="hrelu")
                nc.scalar.activation(
                    out=hrelu[:],
                    in_=t2[:],
                    func=mybir.ActivationFunctionType.Relu,
                )
                pt = pst.tile([P, P], F32, space="PSUM", tag="pt")
                nc.tensor.transpose(out=pt[:], in_=hrelu[:], identity=ident[:])
                hT = epi.tile([P, P], F32, tag="hT")
                nc.scalar.copy(out=hT[:], in_=pt[:])
                if l < 2:
                    pp2 = psp.tile([P, P], F32, space="PSUM", tag="mm")
                    nc.tensor.matmul(
                        out=pp2[:],
                        lhsT=hT[:],
                        rhs=W_sb[l + 1][:],
                        start=True,
                        stop=True,
                    )
                    hs2 = hs_keep[(l + 1) % 2][:, s * P : (s + 1) * P]
                    nc.scalar.activation(
                        out=hs2,
                        in_=pp2[:],
                        func=mybir.ActivationFunctionType.Copy,
                        scale=dinv_sb[:, s : s + 1],
                    )
                    nc.sync.dma_start(
                        out=cc_in[s * P : (s + 1) * P, :], in_=hs2
                    )
                    if (s + 1) % SPB == 0:
                        allgather_block(l + 1, s // SPB)
                else:
                    # MLP head: out = relu(h3@fc1a + qgT) @ fc2 + fc2_b
                    pm = psp.tile([P, P], F32, space="PSUM", tag="mm")
                    nc.tensor.matmul(
                        out=pm[:], lhsT=hT[:], rhs=fc1a_sb[:], start=True, stop=True
                    )
                    u = epi.tile([P, P], F32, tag="u")
                    nc.vector.tensor_tensor(
                        out=u[:],
                        in0=pm[:],
                        in1=qgT_sb[:, s * P : (s + 1) * P],
                        op=mybir.AluOpType.add,
                    )
                    ur = epi.tile([P, P], F32, tag="ur")
                    nc.scalar.activation(
                        out=ur[:], in_=u[:], func=mybir.ActivationFunctionType.Relu
                    )
                    pt2 = pst.tile([P, P], F32, space="PSUM", tag="pt")
                    nc.tensor.transpose(out=pt2[:], in_=ur[:], identity=ident[:])
                    uT = epi.tile([P, P], F32, tag="uT")
                    nc.scalar.copy(out=uT[:], in_=pt2[:])
                    po = psp.tile([P, OUTC], F32, space="PSUM", tag="mm")
                    nc.tensor.matmul(
                        out=po[:], lhsT=uT[:], rhs=fc2w_sb[:], start=True, stop=True
                    )
                    ob = epi.tile([P, OUTC], F32, tag="ob")
                    nc.vector.tensor_tensor(
                        out=ob[:], in0=po[:], in1=fc2bb_sb[:], op=mybir.AluOpType.add
                    )
                    nc.sync.dma_start(
                        out=out_d[s * P : (s + 1) * P, :], in_=ob[:]
                    )
    nc.compile()
    return nc


# ---------------------------------------------------------------- interface
_CACHE = {}


def kernel(**inputs):
    trace = bool(int(os.environ.get("GCN_TRACE", "0")))
    if trace:
        _install_axon_prof()
    import ml_dtypes
    from concourse.bass_utils import run_bass_kernel_spmd

    bf16 = ml_dtypes.bfloat16
    x = np.ascontiguousarray(np.asarray(inputs["x"], dtype=np.float32))
    qe = np.asarray(inputs["question_embedding"], dtype=np.float32)
    pp = preprocess(inputs["edge_index"], inputs["batch"])

    key = (
        tuple(pp["chunks_lo"].tolist()),
        tuple(pp["chunks_hi"].tolist()),
        pp["piece0_bound"],
    )
    if key not in _CACHE:
        _CACHE[key] = build_program(pp)
    nc = _CACHE[key]

    W = [np.asarray(inputs[f"W{i}"], np.float32) for i in range(3)]
    b = [np.asarray(inputs[f"b{i}"], np.float32) for i in range(3)]
    fc0_w = np.asarray(inputs["fc0_w"], np.float32)
    fc0_b = np.asarray(inputs["fc0_b"], np.float32)
    fc1_w = np.asarray(inputs["fc1_w"], np.float32)
    fc1_b = np.asarray(inputs["fc1_b"], np.float32)
    fc2_w = np.asarray(inputs["fc2_w"], np.float32)
    fc2_b = np.asarray(inputs["fc2_b"], np.float32)

    # host question path: qq = relu(qe@fc0 + fc0_b) @ fc1_w[128:] + fc1_b
    q = np.maximum(qe @ fc0_w + fc0_b, 0.0)
    qq = q @ fc1_w[P:] + fc1_b  # [G, 128]

    iota = np.broadcast_to(np.arange(P, dtype=np.float32), (P, P)).astype(bf16)
    common = {
        "iota_in": np.ascontiguousarray(iota),
        "W0": W[0].astype(bf16),
        "W1": W[1],
        "W2": W[2],
        "bb0": np.broadcast_to(b[0], (P, P)).copy(),
        "bb1": np.broadcast_to(b[1], (P, P)).copy(),
        "bb2": np.broadcast_to(b[2], (P, P)).copy(),
        "fc1a": np.ascontiguousarray(fc1_w[:P]),
        "fc2w": fc2_w,
        "fc2bb": np.broadcast_to(fc2_b, (P, OUTC)).copy(),
    }

    in_maps = []
    for c in range(NCORES):
        perm = pp["node_perm"][c]
        valid = perm >= 0
        xTc = np.zeros((P, SLOT_ROWS), dtype=np.float32)
        xTc[:, valid] = x[perm[valid]].T
        # qgT[p, s*128+f] = qq[batch[node(c,s,p)], f]
        gids = pp["gid_slot"][c]  # [P, TPC]
        qgTc = qq[gids].reshape(P, TPC * P).astype(np.float32)
        m = dict(common)
        m["xT"] = xTc.astype(bf16)
        m["idxW"] = np.ascontiguousarray(pp["idxW"][c])
        m["dstin"] = np.ascontiguousarray(pp["dstin_T"][c].astype(bf16))
        m["dinv_in"] = np.ascontiguousarray(pp["dinv_slot"][c])
        m["qgT"] = np.ascontiguousarray(qgTc)
        in_maps.append(m)

    res = run_bass_kernel_spmd(
        nc,
        in_maps,
        list(range(NCORES)),
        trace=trace,
    )
    kernel.last_result = res

    out = np.zeros((N, OUTC), dtype=np.float32)
    for c in range(NCORES):
        perm = pp["node_perm"][c]
        valid = perm >= 0
        out[perm[valid]] = res.results[c]["out"][valid]
    return out


# revision 26
# speedup vs baseline: 1.0007x; 1.0007x over previous
"""Trainium2 Bass kernel for the GCN model (nn_GCNModel_57853209477141).

Model: 3x GCNConv(128->128, sym-norm with self loops) with ReLU, question
embedding MLP, concat, 2-layer MLP head -> [50000, 32].

v2 design (8 NeuronCores, single SPMD launch):
- dst-node sharding: tiles of 128 nodes snake-dealt to cores by edge count;
  one compile-time schedule serves all 8 cores (SPMD).
- GCN norm factorization: agg[v] = dinv[v] * sum_e (dinv*h)[src_e]; tables
  store h~ = dinv*h in bf16, per-edge norm disappears.
- gather primitive: gpsimd.dma_gather (InstDMAGatherAnt), <=8 chunks
  (1024 rows) per instruction, int16 indices wrapped over 16 partitions.
  Table split at row 32768 into lo/hi halves to fit int16 range; each
  slot's edge stream is [lo chunks..., hi chunks...].
- aggregation per chunk: 0/1 one-hot [edge,dst] built on DVE via
  iota-compare, matmul-accumulated (bf16) into PSUM; self-loop term added
  from the resident h~ slice via identity matmul.
- single block-major table layout for all 3 layers; AllGather split into
  7 row-blocks fired as production completes (incl. layer 0) so collective
  traffic overlaps compute/gather.
- question path computed on host (0.1% of FLOPs), expanded per node with
  fc1 bias folded, loaded as a constant.

Host preprocessing: index/layout work + the tiny question MLP; all O(E*F)
and O(N*F*F) float work runs on device.
"""
import os
import sys
import types
from contextlib import ExitStack

import numpy as np

# ---------------------------------------------------------------- constants
N = 50000
E = 800000
G = 64
P = 128
NCORES = 8
TPC = 49  # tile slots per core
SPB = 7   # slots per AllGather block
NBLK = TPC // SPB  # 7
SLOT_ROWS = TPC * P  # 6272
NT = NCORES * SLOT_ROWS  # 50176
QD = 768
OUTC = 32
HALF = 32768  # int16 index split point
MAXC = 8  # max chunks (1024 rows) per dma_gather piece


def _install_axon_prof():
    """Register NTFF profile hook if the image's antenv lacks it; neuter
    bucket upload (zero-egress). Harmless when running without tracing."""
    try:
        from antenv import axon_hooks  # noqa: F401
    except ImportError:
        try:
            import antenv
            from trn_agent_boot.trn_boot import _ntff_profile_via_ctypes

            hook = _ntff_profile_via_ctypes("/opt/axon/libaxon_pjrt.so")
            mod = types.ModuleType("antenv.axon_hooks")
            mod.get_axon_ntff_profile_hook = lambda: hook
            mod.set_axon_ntff_profile_hook = lambda h: None
            sys.modules["antenv.axon_hooks"] = mod
            antenv.axon_hooks = mod
        except Exception:
            pass
    try:
        import concourse.bass_utils as bu

        bu.upload_artifacts = lambda tmpdir: "local://" + str(tmpdir)
    except Exception:
        pass


def _wrap_idxs16(lin: np.ndarray) -> np.ndarray:
    """[n] int -> [128, n//16] int16: wrapped col-major over 16 partitions
    (element i -> [i%16, i//16]), replicated 8x across partition groups."""
    n = lin.shape[0]
    assert n % 16 == 0
    w = np.asarray(lin, dtype=np.int16).reshape(n // 16, 16).T
    return np.tile(w, (8, 1))


# ---------------------------------------------------------------- host prep
def preprocess(edge_index, batch):
    src = np.asarray(edge_index[0], dtype=np.int64)
    dst = np.asarray(edge_index[1], dtype=np.int64)
    deg = (np.bincount(dst, minlength=N) + 1).astype(np.float64)
    dinv = np.where(deg > 0, 1.0 / np.sqrt(deg), 0.0).astype(np.float32)

    n_tiles = (N + P - 1) // P  # 391
    tile_of_node = np.arange(N) // P
    dst_tile = dst // P
    tile_counts = np.bincount(dst_tile, minlength=n_tiles)

    # snake-deal tiles (sorted by edge count desc) across cores
    order_all = np.argsort(-tile_counts, kind="stable")
    core_tiles = [[] for _ in range(NCORES)]
    for r in range(TPC):
        batch_t = order_all[r * NCORES : (r + 1) * NCORES]
        seq = range(NCORES) if r % 2 == 0 else range(NCORES - 1, -1, -1)
        for j, c in enumerate(seq):
            core_tiles[c].append(int(batch_t[j]) if j < len(batch_t) else -1)
    core_of_tile = np.full(n_tiles, -1, dtype=np.int64)
    slot_of_tile = np.full(n_tiles, -1, dtype=np.int64)
    for c in range(NCORES):
        for s, t in enumerate(core_tiles[c]):
            if t >= 0:
                core_of_tile[t] = c
                slot_of_tile[t] = s

    # block-major table row: [block][core][slot%SPB][pos]
    blk = slot_of_tile[tile_of_node] // SPB
    table_row = (
        blk * (NCORES * SPB * P)
        + core_of_tile[tile_of_node] * (SPB * P)
        + (slot_of_tile[tile_of_node] % SPB) * P
        + (np.arange(N) % P)
    )

    order = np.argsort(dst_tile, kind="stable")
    src_sorted = src[order]
    dst_sorted = dst[order]
    sorted_tiles = dst_tile[order]
    tile_starts = np.searchsorted(sorted_tiles, np.arange(n_tiles))
    tile_ends = np.searchsorted(sorted_tiles, np.arange(n_tiles), side="right")

    # per (core, slot): lo/hi edge streams (by table_row of src)
    lo_idx = [[None] * TPC for _ in range(NCORES)]
    hi_idx = [[None] * TPC for _ in range(NCORES)]
    lo_dst = [[None] * TPC for _ in range(NCORES)]
    hi_dst = [[None] * TPC for _ in range(NCORES)]
    for c in range(NCORES):
        for s in range(TPC):
            t = core_tiles[c][s]
            if t < 0:
                lo_idx[c][s] = np.zeros(0, np.int64)
                hi_idx[c][s] = np.zeros(0, np.int64)
                lo_dst[c][s] = np.zeros(0, np.int64)
                hi_dst[c][s] = np.zeros(0, np.int64)
                continue
            a, b = tile_starts[t], tile_ends[t]
            rows = table_row[src_sorted[a:b]]
            din = dst_sorted[a:b] % P
            m = rows < HALF
            # sort lo edges by table row: early gather pieces then only
            # depend on the first AllGather blocks (tighter base bound)
            rl, dl = rows[m], din[m]
            o = np.argsort(rl, kind="stable")
            lo_idx[c][s] = rl[o]
            lo_dst[c][s] = dl[o]
            hi_idx[c][s] = rows[~m] - HALF
            hi_dst[c][s] = din[~m]

    chunks_lo = np.zeros(TPC, dtype=np.int64)
    chunks_hi = np.zeros(TPC, dtype=np.int64)
    for s in range(TPC):
        for c in range(NCORES):
            chunks_lo[s] = max(chunks_lo[s], (len(lo_idx[c][s]) + P - 1) // P)
            chunks_hi[s] = max(chunks_hi[s], (len(hi_idx[c][s]) + P - 1) // P)
    nch = chunks_lo + chunks_hi
    TCH = int(nch.sum())
    TCHL = int(chunks_lo.sum())
    TCHH = int(chunks_hi.sum())
    lo_base = np.cumsum(np.concatenate([[0], chunks_lo[:-1]])).astype(int)
    hi_base = np.cumsum(np.concatenate([[0], chunks_hi[:-1]])).astype(int)

    # per-layer lo/hi chunk streams (slot-major); gather pieces of MAXC
    # chunks cut across slot boundaries so nearly all pieces are full.
    # dstin columns: lo stream at [0, TCHL), hi stream at [TCHL, TCH).
    dstin_T = np.full((NCORES, P, TCH), -1.0, dtype=np.float32)
    idxW = np.zeros((NCORES, P, 8 * TCH), dtype=np.int16)
    piece0_max = []
    for c in range(NCORES):
        lo_lin = np.zeros(TCHL * P, dtype=np.int64)
        lo_dl = np.full(TCHL * P, -1.0, dtype=np.float32)
        hi_lin = np.zeros(TCHH * P, dtype=np.int64)
        hi_dl = np.full(TCHH * P, -1.0, dtype=np.float32)
        for s in range(TPC):
            o = lo_base[s] * P
            nl = len(lo_idx[c][s])
            lo_lin[o : o + nl] = lo_idx[c][s]
            lo_dl[o : o + nl] = lo_dst[c][s]
            o = hi_base[s] * P
            nh = len(hi_idx[c][s])
            hi_lin[o : o + nh] = hi_idx[c][s]
            hi_dl[o : o + nh] = hi_dst[c][s]
        dstin_T[c, :, :TCHL] = lo_dl.reshape(TCHL, P).T
        dstin_T[c, :, TCHL:] = hi_dl.reshape(TCHH, P).T
        if TCHL:
            idxW[c, :, : 8 * TCHL] = _wrap_idxs16(lo_lin)
        if TCHH:
            idxW[c, :, 8 * TCHL :] = _wrap_idxs16(hi_lin)
        piece0_max.append(int(lo_lin[: MAXC * P].max()) if TCHL else 0)

    dinv_slot = np.zeros((NCORES, P, TPC), dtype=np.float32)
    gid_slot = np.zeros((NCORES, P, TPC), dtype=np.int64)
    node_perm = np.full((NCORES, SLOT_ROWS), -1, dtype=np.int64)
    batch = np.asarray(batch, dtype=np.int64)
    for c in range(NCORES):
        for s in range(TPC):
            t = core_tiles[c][s]
            if t < 0:
                continue
            v0 = t * P
            v1 = min(v0 + P, N)
            n = v1 - v0
            dinv_slot[c, :n, s] = dinv[v0:v1]
            gid_slot[c, :n, s] = batch[v0:v1]
            node_perm[c, s * P : s * P + n] = np.arange(v0, v1)

    # piece-0 base bound, quantized up to AG-block rows (block = 7168 rows)
    blk_rows = NCORES * SPB * P
    b0 = max(piece0_max) + 1 if piece0_max else HALF
    piece0_bound = min(-(-b0 // blk_rows) * blk_rows, NT)

    return dict(
        chunks_lo=chunks_lo,
        chunks_hi=chunks_hi,
        piece0_bound=piece0_bound,
        TCH=TCH,
        TCHL=TCHL,
        TCHH=TCHH,
        lo_base=lo_base,
        hi_base=hi_base,
        dstin_T=dstin_T,
        idxW=idxW,
        dinv_slot=dinv_slot,
        gid_slot=gid_slot,
        node_perm=node_perm,
    )


# ------------------------------------------------------------- bass program
def build_program(schedule):
    import concourse.bacc as bacc
    import concourse.bass as bass
    import concourse.tile as tile
    from concourse import mybir
    from concourse.masks import make_identity

    F32 = mybir.dt.float32
    BF16 = mybir.dt.bfloat16
    I16 = mybir.dt.int16
    chunks_lo = schedule["chunks_lo"]
    chunks_hi = schedule["chunks_hi"]
    piece0_bound = schedule["piece0_bound"]
    TCH = schedule["TCH"]
    TCHL = schedule["TCHL"]
    TCHH = schedule["TCHH"]
    lo_base = schedule["lo_base"]
    hi_base = schedule["hi_base"]

    nc = bacc.Bacc(
        "TRN2", target_bir_lowering=False, dynamic_dma_scratch_size=32768
    )
    dp = nc.declare_dram_parameter
    xT = dp("xT", [P, SLOT_ROWS], BF16, isOutput=False)
    idxW_in = dp("idxW", [P, 8 * TCH], I16, isOutput=False)
    dstin = dp("dstin", [P, TCH], BF16, isOutput=False)
    iota_in = dp("iota_in", [P, P], BF16, isOutput=False)
    dinv_in = dp("dinv_in", [P, TPC], F32, isOutput=False)
    W0_in = dp("W0", [P, P], BF16, isOutput=False)
    W1_in = dp("W1", [P, P], F32, isOutput=False)
    W2_in = dp("W2", [P, P], F32, isOutput=False)
    bb_in = [dp(f"bb{i}", [P, P], F32, isOutput=False) for i in range(3)]
    fc1a_in = dp("fc1a", [P, P], F32, isOutput=False)
    fc2w_in = dp("fc2w", [P, OUTC], F32, isOutput=False)
    fc2bb_in = dp("fc2bb", [P, OUTC], F32, isOutput=False)
    qgT_in = dp("qgT", [P, SLOT_ROWS], F32, isOutput=False)
    out_d = dp("out", [SLOT_ROWS, OUTC], F32, isOutput=True)

    cc_in = nc.dram_tensor("cc_in", [SLOT_ROWS, P], BF16)
    tables = [
        nc.dram_tensor(f"table{l}", [NT, P], BF16, addr_space="Shared")
        for l in range(3)
    ]

    # stream descriptors: (dstin col offset, idx col offset, total chunks)
    streams = [(0, 0, TCHL), (TCHL, 8 * TCHL, TCHH)]

    with tile.TileContext(nc) as tc, ExitStack() as ctx:
        const = ctx.enter_context(tc.tile_pool(name="const", bufs=1))
        gp = ctx.enter_context(tc.tile_pool(name="gp", bufs=9))
        ohp = ctx.enter_context(tc.tile_pool(name="ohp", bufs=8))
        psagg = ctx.enter_context(tc.tile_pool(name="psagg", bufs=3, space="PSUM"))
        psp = ctx.enter_context(tc.tile_pool(name="psp", bufs=3, space="PSUM"))
        pst = ctx.enter_context(tc.tile_pool(name="pst", bufs=2, space="PSUM"))
        epi = ctx.enter_context(tc.tile_pool(name="epi", bufs=3))

        # ---- constants (W0/dinv then xT in AG-block slices: production of
        # block 0 starts after ~230 KB instead of the whole 1.6 MB load)
        W0_sb = const.tile([P, P], BF16)
        nc.sync.dma_start(out=W0_sb[:], in_=W0_in[:])
        dinv_sb = const.tile([P, TPC], F32)
        nc.sync.dma_start(out=dinv_sb[:], in_=dinv_in[:])
        xT_sb = const.tile([P, SLOT_ROWS], BF16)
        for j in range(NBLK):
            c0, c1 = j * SPB * P, (j + 1) * SPB * P
            nc.sync.dma_start(out=xT_sb[:, c0:c1], in_=xT[:, c0:c1])

        # resident own-slice h~ buffers (self-loop source), layer parity
        hs_keep = [
            const.tile([P, SLOT_ROWS], BF16, tag=f"hsk{i}", name=f"hsk{i}")
            for i in range(2)
        ]

        def allgather_block(l, j):
            r0 = j * SPB * P
            r1 = (j + 1) * SPB * P
            nc.gpsimd.collective_compute(
                "AllGather",
                mybir.AluOpType.bypass,
                replica_groups=[list(range(NCORES))],
                ins=[cc_in[r0:r1].opt()],
                outs=[
                    tables[l][
                        j * NCORES * SPB * P : (j + 1) * NCORES * SPB * P
                    ].opt()
                ],
            )

        # ---- layer 0 production: h~0 = dinv * (x @ W0), AG fired per block
        for s in range(TPC):
            pp = psp.tile([P, P], F32, space="PSUM", tag="mm")
            nc.tensor.matmul(
                out=pp[:],
                lhsT=xT_sb[:, s * P : (s + 1) * P],
                rhs=W0_sb[:],
                start=True,
                stop=True,
            )
            hs = hs_keep[0][:, s * P : (s + 1) * P]
            nc.scalar.activation(
                out=hs,
                in_=pp[:],
                func=mybir.ActivationFunctionType.Copy,
                scale=dinv_sb[:, s : s + 1],
            )
            nc.sync.dma_start(out=cc_in[s * P : (s + 1) * P, :], in_=hs)
            if (s + 1) % SPB == 0:
                allgather_block(0, s // SPB)

        # ---- remaining constants: emitted AFTER production so their DMAs
        # don't occupy the sync/scalar queues ahead of production's
        # activations and cc_in writes (first consumer is at first gather)
        idxW_sb = const.tile([P, 8 * TCH], I16)
        nc.scalar.dma_start(out=idxW_sb[:], in_=idxW_in[:])
        dstin_sb = const.tile([P, TCH], BF16)
        nc.scalar.dma_start(out=dstin_sb[:], in_=dstin[:])
        iota_sb = const.tile([P, P], BF16)
        nc.sync.dma_start(out=iota_sb[:], in_=iota_in[:])
        W_sb = [None] * 3
        for i, win in ((1, W1_in), (2, W2_in)):
            w = const.tile([P, P], F32, tag=f"W{i}")
            nc.sync.dma_start(out=w[:], in_=win[:])
            W_sb[i] = w
        bb_sb = []
        for i in range(3):
            b = const.tile([P, P], F32, tag=f"bb{i}")
            nc.sync.dma_start(out=b[:], in_=bb_in[i][:])
            bb_sb.append(b)
        fc1a_sb = const.tile([P, P], F32)
        nc.sync.dma_start(out=fc1a_sb[:], in_=fc1a_in[:])
        fc2w_sb = const.tile([P, OUTC], F32)
        nc.sync.dma_start(out=fc2w_sb[:], in_=fc2w_in[:])
        fc2bb_sb = const.tile([P, OUTC], F32)
        nc.sync.dma_start(out=fc2bb_sb[:], in_=fc2bb_in[:])
        qgT_sb = const.tile([P, SLOT_ROWS], F32)
        nc.scalar.dma_start(out=qgT_sb[:], in_=qgT_in[:])
        ident = const.tile([P, P], F32)
        make_identity(nc, ident[:])
        ident_r = const.tile([P, P], BF16, tag="ident_r")
        nc.vector.tensor_copy(out=ident_r[:], in_=ident[:])

        # lo indices are < HALF, i.e. within AG blocks 0-4; bounding the
        # base AP there lets lo gathers start before AG blocks 5-6 land.
        LO_BOUND = 5 * NCORES * SPB * P  # 35840 >= HALF

        # ---- 3 aggregation layers
        for l in range(3):
            table = tables[l]
            piece_tiles = [{}, {}]  # per stream: piece id -> gather tile
            n_lo_pieces = (TCHL + MAXC - 1) // MAXC
            lo_issued = [0]

            def get_piece(h, pj):
                if pj in piece_tiles[h]:
                    return piece_tiles[h][pj]
                _, coloff, tot = streams[h]
                pc = min(MAXC, tot - MAXC * pj)
                g = gp.tile([P, MAXC * P], BF16, tag="g")
                if h == 0:
                    lb = piece0_bound if pj == 0 else LO_BOUND
                    base = table[:lb, :]
                else:
                    base = table[HALF:, :]
                colbase = coloff + 8 * MAXC * pj
                nc.gpsimd.dma_gather(
                    g[:, : pc * P].rearrange("p (k c) -> p k c", k=pc),
                    base,
                    idxW_sb[:, colbase : colbase + pc * 8],
                    pc * P,
                    pc * P,
                    P,
                )
                piece_tiles[h][pj] = g
                return g

            def prefetch_lo(upto):
                while lo_issued[0] < min(upto, n_lo_pieces):
                    get_piece(0, lo_issued[0])
                    lo_issued[0] += 1

            for s in range(TPC):
                prefetch_lo(int(lo_base[s]) // MAXC + 4)
                ps = psagg.tile([P, P], F32, space="PSUM", tag="agg")
                first = True
                for h, b0, cnt in (
                    (0, int(lo_base[s]), int(chunks_lo[s])),
                    (1, int(hi_base[s]), int(chunks_hi[s])),
                ):
                    doff = streams[h][0]
                    for k in range(cnt):
                        kk = b0 + k
                        g = get_piece(h, kk // MAXC)
                        kp = kk % MAXC
                        oh = ohp.tile([P, P], BF16, tag="oh")
                        nc.vector.tensor_tensor(
                            out=oh[:],
                            in0=dstin_sb[
                                :, doff + kk : doff + kk + 1
                            ].to_broadcast([P, P]),
                            in1=iota_sb[:],
                            op=mybir.AluOpType.is_equal,
                        )
                        nc.tensor.matmul(
                            out=ps[:],
                            lhsT=oh[:],
                            rhs=g[:, kp * P : (kp + 1) * P],
                            start=first,
                            stop=False,
                        )
                        first = False
                # self-loop term
                nc.tensor.matmul(
                    out=ps[:],
                    lhsT=ident_r[:],
                    rhs=hs_keep[l % 2][:, s * P : (s + 1) * P],
                    start=first,
                    stop=True,
                )
                # epilogue: h = relu(dinv*agg + b)
                t1 = epi.tile([P, P], F32, tag="t1")
                nc.scalar.activation(
                    out=t1[:],
                    in_=ps[:],
                    func=mybir.ActivationFunctionType.Copy,
                    scale=dinv_sb[:, s : s + 1],
                )
                t2 = epi.tile([P, P], F32, tag="t2")
                nc.vector.tensor_tensor(
                    out=t2[:], in0=t1[:], in1=bb_sb[l][:], op=mybir.AluOpType.add
                )
                hrelu = epi.tile([P, P], F32, tag="hrelu")
                nc.scalar.activation(
                    out=hrelu[:],
                    in_=t2[:],
                    func=mybir.ActivationFunctionType.Relu,
                )
                pt = pst.tile([P, P], F32, space="PSUM", tag="pt")
                nc.tensor.transpose(out=pt[:], in_=hrelu[:], identity=ident[:])
                hT = epi.tile([P, P], F32, tag="hT")
                nc.scalar.copy(out=hT[:], in_=pt[:])
                if l < 2:
                    pp2 = psp.tile([P, P], F32, space="PSUM", tag="mm")
                    nc.tensor.matmul(
                        out=pp2[:],
                        lhsT=hT[:],
                        rhs=W_sb[l + 1][:],
                        start=True,
                        stop=True,
                    )
                    hs2 = hs_keep[(l + 1) % 2][:, s * P : (s + 1) * P]
                    nc.scalar.activation(
                        out=hs2,
                        in_=pp2[:],
                        func=mybir.ActivationFunctionType.Copy,
                        scale=dinv_sb[:, s : s + 1],
                    )
                    nc.sync.dma_start(
                        out=cc_in[s * P : (s + 1) * P, :], in_=hs2
                    )
                    if (s + 1) % SPB == 0:
                        allgather_block(l + 1, s // SPB)
                else:
                    # MLP head: out = relu(h3@fc1a + qgT) @ fc2 + fc2_b
                    pm = psp.tile([P, P], F32, space="PSUM", tag="mm")
                    nc.tensor.matmul(
                        out=pm[:], lhsT=hT[:], rhs=fc1a_sb[:], start=True, stop=True
                    )
                    u = epi.tile([P, P], F32, tag="u")
                    nc.vector.tensor_tensor(
                        out=u[:],
                        in0=pm[:],
                        in1=qgT_sb[:, s * P : (s + 1) * P],
                        op=mybir.AluOpType.add,
                    )
                    ur = epi.tile([P, P], F32, tag="ur")
                    nc.scalar.activation(
                        out=ur[:], in_=u[:], func=mybir.ActivationFunctionType.Relu
                    )
                    pt2 = pst.tile([P, P], F32, space="PSUM", tag="pt")
                    nc.tensor.transpose(out=pt2[:], in_=ur[:], identity=ident[:])
                    uT = epi.tile([P, P], F32, tag="uT")
                    nc.scalar.copy(out=uT[:], in_=pt2[:])
                    po = psp.tile([P, OUTC], F32, space="PSUM", tag="mm")
                    nc.tensor.matmul(
                        out=po[:], lhsT=uT[:], rhs=fc2w_sb[:], start=True, stop=True
                    )
                    ob = epi.tile([P, OUTC], F32, tag="ob")
                    nc.vector.tensor_tensor(
                        out=ob[:], in0=po[:], in1=fc2bb_sb[:], op=mybir.AluOpType.add
                    )
                    nc.sync.dma_start(
                        out=out_d[s * P : (s + 1) * P, :], in_=ob[:]
                    )
    nc.compile()
    return nc


# ---------------------------------------------------------------- interface
_CACHE = {}


def kernel(**inputs):
    trace = bool(int(os.environ.get("GCN_TRACE", "0")))
    if trace:
        _install_axon_prof()
    import ml_dtypes
    from concourse.bass_utils import run_bass_kernel_spmd

    bf16 = ml_dtypes.bfloat16
    x = np.ascontiguousarray(np.asarray(inputs["x"], dtype=np.float32))
    qe = np.asarray(inputs["question_embedding"], dtype=np.float32)
    pp = preprocess(inputs["edge_index"], inputs["batch"])

    key = (
        tuple(pp["chunks_lo"].tolist()),
        tuple(pp["chunks_hi"].tolist()),
        pp["piece0_bound"],
    )
    if key not in _CACHE:
        _CACHE[key] = build_program(pp)
    nc = _CACHE[key]

    W = [np.asarray(inputs[f"W{i}"], np.float32) for i in range(3)]
    b = [np.asarray(inputs[f"b{i}"], np.float32) for i in range(3)]
    fc0_w = np.asarray(inputs["fc0_w"], np.float32)
    fc0_b = np.asarray(inputs["fc0_b"], np.float32)
    fc1_w = np.asarray(inputs["fc1_w"], np.float32)
    fc1_b = np.asarray(inputs["fc1_b"], np.float32)
    fc2_w = np.asarray(inputs["fc2_w"], np.float32)
    fc2_b = np.asarray(inputs["fc2_b"], np.float32)

    # host question path: qq = relu(qe@fc0 + fc0_b) @ fc1_w[128:] + fc1_b
    q = np.maximum(qe @ fc0_w + fc0_b, 0.0)
    qq = q @ fc1_w[P:] + fc1_b  # [G, 128]

    iota = np.broadcast_to(np.arange(P, dtype=np.float32), (P, P)).astype(bf16)
    common = {
        "iota_in": np.ascontiguousarray(iota),
        "W0": W[0].astype(bf16),
        "W1": W[1],
        "W2": W[2],
        "bb0": np.broadcast_to(b[0], (P, P)).copy(),
        "bb1": np.broadcast_to(b[1], (P, P)).copy(),
        "bb2": np.broadcast_to(b[2], (P, P)).copy(),
        "fc1a": np.ascontiguousarray(fc1_w[:P]),
        "fc2w": fc2_w,
        "fc2bb": np.broadcast_to(fc2_b, (P, OUTC)).copy(),
    }

    in_maps = []
    for c in range(NCORES):
        perm = pp["node_perm"][c]
        valid = perm >= 0
        xTc = np.zeros((P, SLOT_ROWS), dtype=np.float32)
        xTc[:, valid] = x[perm[valid]].T
        # qgT[p, s*128+f] = qq[batch[node(c,s,p)], f]
        gids = pp["gid_slot"][c]  # [P, TPC]
        qgTc = qq[gids].reshape(P, TPC * P).astype(np.float32)
        m = dict(common)
        m["xT"] = xTc.astype(bf16)
        m["idxW"] = np.ascontiguousarray(pp["idxW"][c])
        m["dstin"] = np.ascontiguousarray(pp["dstin_T"][c].astype(bf16))
        m["dinv_in"] = np.ascontiguousarray(pp["dinv_slot"][c])
        m["qgT"] = np.ascontiguousarray(qgTc)
        in_maps.append(m)

    res = run_bass_kernel_spmd(
        nc,
        in_maps,
        list(range(NCORES)),
        trace=trace,
    )
    kernel.last_result = res

    out = np.zeros((N, OUTC), dtype=np.float32)
    for c in range(NCORES):
        perm = pp["node_perm"][c]
        valid = perm >= 0
        out[perm[valid]] = res.results[c]["out"][valid]
    return out


# revision 28
# speedup vs baseline: 1.0218x; 1.0211x over previous
"""Trainium2 Bass kernel for the GCN model (nn_GCNModel_57853209477141).

Model: 3x GCNConv(128->128, sym-norm with self loops) with ReLU, question
embedding MLP, concat, 2-layer MLP head -> [50000, 32].

v2 design (8 NeuronCores, single SPMD launch):
- dst-node sharding: tiles of 128 nodes snake-dealt to cores by edge count;
  one compile-time schedule serves all 8 cores (SPMD).
- GCN norm factorization: agg[v] = dinv[v] * sum_e (dinv*h)[src_e]; tables
  store h~ = dinv*h in bf16, per-edge norm disappears.
- gather primitive: gpsimd.dma_gather (InstDMAGatherAnt), <=8 chunks
  (1024 rows) per instruction, int16 indices wrapped over 16 partitions.
  Table split at row 32768 into lo/hi halves to fit int16 range; each
  slot's edge stream is [lo chunks..., hi chunks...].
- aggregation per chunk: 0/1 one-hot [edge,dst] built on DVE via
  iota-compare, matmul-accumulated (bf16) into PSUM; self-loop term added
  from the resident h~ slice via identity matmul.
- single block-major table layout for all 3 layers; AllGather split into
  7 row-blocks fired as production completes (incl. layer 0) so collective
  traffic overlaps compute/gather.
- question path computed on host (0.1% of FLOPs), expanded per node with
  fc1 bias folded, loaded as a constant.

Host preprocessing: index/layout work + the tiny question MLP; all O(E*F)
and O(N*F*F) float work runs on device.
"""
import os
import sys
import types
from contextlib import ExitStack

import numpy as np

# ---------------------------------------------------------------- constants
N = 50000
E = 800000
G = 64
P = 128
NCORES = 8
TPC = 49  # tile slots per core
SPB = 7   # slots per AllGather block
NBLK = TPC // SPB  # 7
SLOT_ROWS = TPC * P  # 6272
NT = NCORES * SLOT_ROWS  # 50176
QD = 768
OUTC = 32
HALF = 32768  # int16 index split point
MAXC = 8  # max chunks (1024 rows) per dma_gather piece


def _install_axon_prof():
    """Register NTFF profile hook if the image's antenv lacks it; neuter
    bucket upload (zero-egress). Harmless when running without tracing."""
    try:
        from antenv import axon_hooks  # noqa: F401
    except ImportError:
        try:
            import antenv
            from trn_agent_boot.trn_boot import _ntff_profile_via_ctypes

            hook = _ntff_profile_via_ctypes("/opt/axon/libaxon_pjrt.so")
            mod = types.ModuleType("antenv.axon_hooks")
            mod.get_axon_ntff_profile_hook = lambda: hook
            mod.set_axon_ntff_profile_hook = lambda h: None
            sys.modules["antenv.axon_hooks"] = mod
            antenv.axon_hooks = mod
        except Exception:
            pass
    try:
        import concourse.bass_utils as bu

        bu.upload_artifacts = lambda tmpdir: "local://" + str(tmpdir)
    except Exception:
        pass


def _wrap_idxs16(lin: np.ndarray) -> np.ndarray:
    """[n] int -> [128, n//16] int16: wrapped col-major over 16 partitions
    (element i -> [i%16, i//16]), replicated 8x across partition groups."""
    n = lin.shape[0]
    assert n % 16 == 0
    w = np.asarray(lin, dtype=np.int16).reshape(n // 16, 16).T
    return np.tile(w, (8, 1))


# ---------------------------------------------------------------- host prep
def preprocess(edge_index, batch):
    src = np.asarray(edge_index[0], dtype=np.int64)
    dst = np.asarray(edge_index[1], dtype=np.int64)
    deg = (np.bincount(dst, minlength=N) + 1).astype(np.float64)
    dinv = np.where(deg > 0, 1.0 / np.sqrt(deg), 0.0).astype(np.float32)

    n_tiles = (N + P - 1) // P  # 391
    tile_of_node = np.arange(N) // P
    dst_tile = dst // P
    tile_counts = np.bincount(dst_tile, minlength=n_tiles)

    # snake-deal tiles (sorted by edge count desc) across cores
    order_all = np.argsort(-tile_counts, kind="stable")
    core_tiles = [[] for _ in range(NCORES)]
    for r in range(TPC):
        batch_t = order_all[r * NCORES : (r + 1) * NCORES]
        seq = range(NCORES) if r % 2 == 0 else range(NCORES - 1, -1, -1)
        for j, c in enumerate(seq):
            core_tiles[c].append(int(batch_t[j]) if j < len(batch_t) else -1)
    core_of_tile = np.full(n_tiles, -1, dtype=np.int64)
    slot_of_tile = np.full(n_tiles, -1, dtype=np.int64)
    for c in range(NCORES):
        for s, t in enumerate(core_tiles[c]):
            if t >= 0:
                core_of_tile[t] = c
                slot_of_tile[t] = s

    # block-major table row: [block][core][slot%SPB][pos]
    blk = slot_of_tile[tile_of_node] // SPB
    table_row = (
        blk * (NCORES * SPB * P)
        + core_of_tile[tile_of_node] * (SPB * P)
        + (slot_of_tile[tile_of_node] % SPB) * P
        + (np.arange(N) % P)
    )

    order = np.argsort(dst_tile, kind="stable")
    src_sorted = src[order]
    dst_sorted = dst[order]
    sorted_tiles = dst_tile[order]
    tile_starts = np.searchsorted(sorted_tiles, np.arange(n_tiles))
    tile_ends = np.searchsorted(sorted_tiles, np.arange(n_tiles), side="right")

    # per (core, slot): lo/hi edge streams (by table_row of src)
    lo_idx = [[None] * TPC for _ in range(NCORES)]
    hi_idx = [[None] * TPC for _ in range(NCORES)]
    lo_dst = [[None] * TPC for _ in range(NCORES)]
    hi_dst = [[None] * TPC for _ in range(NCORES)]
    for c in range(NCORES):
        for s in range(TPC):
            t = core_tiles[c][s]
            if t < 0:
                lo_idx[c][s] = np.zeros(0, np.int64)
                hi_idx[c][s] = np.zeros(0, np.int64)
                lo_dst[c][s] = np.zeros(0, np.int64)
                hi_dst[c][s] = np.zeros(0, np.int64)
                continue
            a, b = tile_starts[t], tile_ends[t]
            rows = table_row[src_sorted[a:b]]
            din = dst_sorted[a:b] % P
            m = rows < HALF
            # sort lo edges by table row: early gather pieces then only
            # depend on the first AllGather blocks (tighter base bound)
            rl, dl = rows[m], din[m]
            o = np.argsort(rl, kind="stable")
            lo_idx[c][s] = rl[o]
            lo_dst[c][s] = dl[o]
            hi_idx[c][s] = rows[~m] - HALF
            hi_dst[c][s] = din[~m]

    chunks_lo = np.zeros(TPC, dtype=np.int64)
    chunks_hi = np.zeros(TPC, dtype=np.int64)
    for s in range(TPC):
        for c in range(NCORES):
            chunks_lo[s] = max(chunks_lo[s], (len(lo_idx[c][s]) + P - 1) // P)
            chunks_hi[s] = max(chunks_hi[s], (len(hi_idx[c][s]) + P - 1) // P)
    nch = chunks_lo + chunks_hi
    TCH = int(nch.sum())
    TCHL = int(chunks_lo.sum())
    TCHH = int(chunks_hi.sum())
    lo_base = np.cumsum(np.concatenate([[0], chunks_lo[:-1]])).astype(int)
    hi_base = np.cumsum(np.concatenate([[0], chunks_hi[:-1]])).astype(int)

    # per-layer lo/hi chunk streams (slot-major); gather pieces of MAXC
    # chunks cut across slot boundaries so nearly all pieces are full.
    # dstin columns: lo stream at [0, TCHL), hi stream at [TCHL, TCH).
    dstin_T = np.full((NCORES, P, TCH), -1.0, dtype=np.float32)
    idxW = np.zeros((NCORES, P, 8 * TCH), dtype=np.int16)
    piece0_max = []
    for c in range(NCORES):
        lo_lin = np.zeros(TCHL * P, dtype=np.int64)
        lo_dl = np.full(TCHL * P, -1.0, dtype=np.float32)
        hi_lin = np.zeros(TCHH * P, dtype=np.int64)
        hi_dl = np.full(TCHH * P, -1.0, dtype=np.float32)
        for s in range(TPC):
            o = lo_base[s] * P
            nl = len(lo_idx[c][s])
            lo_lin[o : o + nl] = lo_idx[c][s]
            lo_dl[o : o + nl] = lo_dst[c][s]
            o = hi_base[s] * P
            nh = len(hi_idx[c][s])
            hi_lin[o : o + nh] = hi_idx[c][s]
            hi_dl[o : o + nh] = hi_dst[c][s]
        dstin_T[c, :, :TCHL] = lo_dl.reshape(TCHL, P).T
        dstin_T[c, :, TCHL:] = hi_dl.reshape(TCHH, P).T
        if TCHL:
            idxW[c, :, : 8 * TCHL] = _wrap_idxs16(lo_lin)
        if TCHH:
            idxW[c, :, 8 * TCHL :] = _wrap_idxs16(hi_lin)
        piece0_max.append(int(lo_lin[: MAXC * P].max()) if TCHL else 0)

    dinv_slot = np.zeros((NCORES, P, TPC), dtype=np.float32)
    gid_slot = np.zeros((NCORES, P, TPC), dtype=np.int64)
    node_perm = np.full((NCORES, SLOT_ROWS), -1, dtype=np.int64)
    batch = np.asarray(batch, dtype=np.int64)
    for c in range(NCORES):
        for s in range(TPC):
            t = core_tiles[c][s]
            if t < 0:
                continue
            v0 = t * P
            v1 = min(v0 + P, N)
            n = v1 - v0
            dinv_slot[c, :n, s] = dinv[v0:v1]
            gid_slot[c, :n, s] = batch[v0:v1]
            node_perm[c, s * P : s * P + n] = np.arange(v0, v1)

    # piece-0 base bound, quantized up to AG-block rows (block = 7168 rows)
    blk_rows = NCORES * SPB * P
    b0 = max(piece0_max) + 1 if piece0_max else HALF
    piece0_bound = min(-(-b0 // blk_rows) * blk_rows, NT)

    return dict(
        chunks_lo=chunks_lo,
        chunks_hi=chunks_hi,
        piece0_bound=piece0_bound,
        TCH=TCH,
        TCHL=TCHL,
        TCHH=TCHH,
        lo_base=lo_base,
        hi_base=hi_base,
        dstin_T=dstin_T,
        idxW=idxW,
        dinv_slot=dinv_slot,
        gid_slot=gid_slot,
        node_perm=node_perm,
    )


# ------------------------------------------------------------- bass program
def build_program(schedule):
    import concourse.bacc as bacc
    import concourse.bass as bass
    import concourse.tile as tile
    from concourse import mybir
    from concourse.masks import make_identity

    F32 = mybir.dt.float32
    BF16 = mybir.dt.bfloat16
    I16 = mybir.dt.int16
    chunks_lo = schedule["chunks_lo"]
    chunks_hi = schedule["chunks_hi"]
    piece0_bound = schedule["piece0_bound"]
    TCH = schedule["TCH"]
    TCHL = schedule["TCHL"]
    TCHH = schedule["TCHH"]
    lo_base = schedule["lo_base"]
    hi_base = schedule["hi_base"]

    nc = bacc.Bacc(
        "TRN2", target_bir_lowering=False, dynamic_dma_scratch_size=32768
    )
    dp = nc.declare_dram_parameter
    xT = dp("xT", [P, SLOT_ROWS], BF16, isOutput=False)
    idxW_in = dp("idxW", [P, 8 * TCH], I16, isOutput=False)
    dstin = dp("dstin", [P, TCH], BF16, isOutput=False)
    iota_in = dp("iota_in", [P, P], BF16, isOutput=False)
    dinv_in = dp("dinv_in", [P, TPC], F32, isOutput=False)
    W0_in = dp("W0", [P, P], BF16, isOutput=False)
    W1_in = dp("W1", [P, P], F32, isOutput=False)
    W2_in = dp("W2", [P, P], F32, isOutput=False)
    bb_in = [dp(f"bb{i}", [P, P], F32, isOutput=False) for i in range(3)]
    fc1a_in = dp("fc1a", [P, P], F32, isOutput=False)
    fc2w_in = dp("fc2w", [P, OUTC], F32, isOutput=False)
    fc2bb_in = dp("fc2bb", [P, OUTC], F32, isOutput=False)
    qgT_in = dp("qgT", [P, SLOT_ROWS], F32, isOutput=False)
    out_d = dp("out", [SLOT_ROWS, OUTC], F32, isOutput=True)

    cc_in = nc.dram_tensor("cc_in", [SLOT_ROWS, P], BF16)
    tables = [
        nc.dram_tensor(f"table{l}", [NT, P], BF16, addr_space="Shared")
        for l in range(3)
    ]

    # stream descriptors: (dstin col offset, idx col offset, total chunks)
    streams = [(0, 0, TCHL), (TCHL, 8 * TCHL, TCHH)]

    with tile.TileContext(nc) as tc, ExitStack() as ctx:
        const = ctx.enter_context(tc.tile_pool(name="const", bufs=1))
        gp = ctx.enter_context(tc.tile_pool(name="gp", bufs=12))
        ohp = ctx.enter_context(tc.tile_pool(name="ohp", bufs=8))
        psagg = ctx.enter_context(tc.tile_pool(name="psagg", bufs=3, space="PSUM"))
        psp = ctx.enter_context(tc.tile_pool(name="psp", bufs=3, space="PSUM"))
        pst = ctx.enter_context(tc.tile_pool(name="pst", bufs=2, space="PSUM"))
        epi = ctx.enter_context(tc.tile_pool(name="epi", bufs=3))

        # ---- constants (W0/dinv then xT in AG-block slices: production of
        # block 0 starts after ~230 KB instead of the whole 1.6 MB load)
        W0_sb = const.tile([P, P], BF16)
        nc.sync.dma_start(out=W0_sb[:], in_=W0_in[:])
        dinv_sb = const.tile([P, TPC], F32)
        nc.sync.dma_start(out=dinv_sb[:], in_=dinv_in[:])
        xT_sb = const.tile([P, SLOT_ROWS], BF16)
        for j in range(NBLK):
            c0, c1 = j * SPB * P, (j + 1) * SPB * P
            nc.sync.dma_start(out=xT_sb[:, c0:c1], in_=xT[:, c0:c1])

        # resident own-slice h~ buffers (self-loop source), layer parity
        hs_keep = [
            const.tile([P, SLOT_ROWS], BF16, tag=f"hsk{i}", name=f"hsk{i}")
            for i in range(2)
        ]

        def allgather_block(l, j):
            r0 = j * SPB * P
            r1 = (j + 1) * SPB * P
            nc.gpsimd.collective_compute(
                "AllGather",
                mybir.AluOpType.bypass,
                replica_groups=[list(range(NCORES))],
                ins=[cc_in[r0:r1].opt()],
                outs=[
                    tables[l][
                        j * NCORES * SPB * P : (j + 1) * NCORES * SPB * P
                    ].opt()
                ],
            )

        # ---- layer 0 production: h~0 = dinv * (x @ W0), AG fired per block
        for s in range(TPC):
            pp = psp.tile([P, P], F32, space="PSUM", tag="mm")
            nc.tensor.matmul(
                out=pp[:],
                lhsT=xT_sb[:, s * P : (s + 1) * P],
                rhs=W0_sb[:],
                start=True,
                stop=True,
            )
            hs = hs_keep[0][:, s * P : (s + 1) * P]
            nc.scalar.activation(
                out=hs,
                in_=pp[:],
                func=mybir.ActivationFunctionType.Copy,
                scale=dinv_sb[:, s : s + 1],
            )
            nc.sync.dma_start(out=cc_in[s * P : (s + 1) * P, :], in_=hs)
            if (s + 1) % SPB == 0:
                allgather_block(0, s // SPB)

        # ---- remaining constants: emitted AFTER production so their DMAs
        # don't occupy the sync/scalar queues ahead of production's
        # activations and cc_in writes (first consumer is at first gather)
        idxW_sb = const.tile([P, 8 * TCH], I16)
        nc.scalar.dma_start(out=idxW_sb[:], in_=idxW_in[:])
        dstin_sb = const.tile([P, TCH], BF16)
        nc.scalar.dma_start(out=dstin_sb[:], in_=dstin[:])
        iota_sb = const.tile([P, P], BF16)
        nc.sync.dma_start(out=iota_sb[:], in_=iota_in[:])
        W_sb = [None] * 3
        for i, win in ((1, W1_in), (2, W2_in)):
            w = const.tile([P, P], F32, tag=f"W{i}")
            nc.sync.dma_start(out=w[:], in_=win[:])
            W_sb[i] = w
        bb_sb = []
        for i in range(3):
            b = const.tile([P, P], F32, tag=f"bb{i}")
            nc.sync.dma_start(out=b[:], in_=bb_in[i][:])
            bb_sb.append(b)
        fc1a_sb = const.tile([P, P], F32)
        nc.sync.dma_start(out=fc1a_sb[:], in_=fc1a_in[:])
        fc2w_sb = const.tile([P, OUTC], F32)
        nc.sync.dma_start(out=fc2w_sb[:], in_=fc2w_in[:])
        fc2bb_sb = const.tile([P, OUTC], F32)
        nc.sync.dma_start(out=fc2bb_sb[:], in_=fc2bb_in[:])
        qgT_sb = const.tile([P, SLOT_ROWS], F32)
        nc.scalar.dma_start(out=qgT_sb[:], in_=qgT_in[:])
        ident = const.tile([P, P], F32)
        make_identity(nc, ident[:])
        ident_r = const.tile([P, P], BF16, tag="ident_r")
        nc.vector.tensor_copy(out=ident_r[:], in_=ident[:])

        # lo indices are < HALF, i.e. within AG blocks 0-4; bounding the
        # base AP there lets lo gathers start before AG blocks 5-6 land.
        LO_BOUND = 5 * NCORES * SPB * P  # 35840 >= HALF

        # ---- 3 aggregation layers
        for l in range(3):
            table = tables[l]
            piece_tiles = [{}, {}]  # per stream: piece id -> gather tile
            n_lo_pieces = (TCHL + MAXC - 1) // MAXC
            lo_issued = [0]

            def get_piece(h, pj):
                if pj in piece_tiles[h]:
                    return piece_tiles[h][pj]
                _, coloff, tot = streams[h]
                pc = min(MAXC, tot - MAXC * pj)
                g = gp.tile([P, MAXC * P], BF16, tag="g")
                if h == 0:
                    lb = piece0_bound if pj == 0 else LO_BOUND
                    base = table[:lb, :]
                else:
                    base = table[HALF:, :]
                colbase = coloff + 8 * MAXC * pj
                nc.gpsimd.dma_gather(
                    g[:, : pc * P].rearrange("p (k c) -> p k c", k=pc),
                    base,
                    idxW_sb[:, colbase : colbase + pc * 8],
                    pc * P,
                    pc * P,
                    P,
                )
                piece_tiles[h][pj] = g
                return g

            def prefetch_lo(upto):
                while lo_issued[0] < min(upto, n_lo_pieces):
                    get_piece(0, lo_issued[0])
                    lo_issued[0] += 1

            for s in range(TPC):
                prefetch_lo(int(lo_base[s]) // MAXC + 8)
                ps = psagg.tile([P, P], F32, space="PSUM", tag="agg")
                first = True
                for h, b0, cnt in (
                    (0, int(lo_base[s]), int(chunks_lo[s])),
                    (1, int(hi_base[s]), int(chunks_hi[s])),
                ):
                    doff = streams[h][0]
                    for k in range(cnt):
                        kk = b0 + k
                        g = get_piece(h, kk // MAXC)
                        kp = kk % MAXC
                        oh = ohp.tile([P, P], BF16, tag="oh")
                        nc.vector.tensor_tensor(
                            out=oh[:],
                            in0=dstin_sb[
                                :, doff + kk : doff + kk + 1
                            ].to_broadcast([P, P]),
                            in1=iota_sb[:],
                            op=mybir.AluOpType.is_equal,
                        )
                        nc.tensor.matmul(
                            out=ps[:],
                            lhsT=oh[:],
                            rhs=g[:, kp * P : (kp + 1) * P],
                            start=first,
                            stop=False,
                        )
                        first = False
                # self-loop term
                nc.tensor.matmul(
                    out=ps[:],
                    lhsT=ident_r[:],
                    rhs=hs_keep[l % 2][:, s * P : (s + 1) * P],
                    start=first,
                    stop=True,
                )
                # epilogue: h = relu(dinv*agg + b)
                t1 = epi.tile([P, P], F32, tag="t1")
                nc.scalar.activation(
                    out=t1[:],
                    in_=ps[:],
                    func=mybir.ActivationFunctionType.Copy,
                    scale=dinv_sb[:, s : s + 1],
                )
                t2 = epi.tile([P, P], F32, tag="t2")
                nc.vector.tensor_tensor(
                    out=t2[:], in0=t1[:], in1=bb_sb[l][:], op=mybir.AluOpType.add
                )
                hrelu = epi.tile([P, P], F32, tag="hrelu")
                nc.scalar.activation(
                    out=hrelu[:],
                    in_=t2[:],
                    func=mybir.ActivationFunctionType.Relu,
                )
                pt = pst.tile([P, P], F32, space="PSUM", tag="pt")
                nc.tensor.transpose(out=pt[:], in_=hrelu[:], identity=ident[:])
                hT = epi.tile([P, P], F32, tag="hT")
                nc.scalar.copy(out=hT[:], in_=pt[:])
                if l < 2:
                    pp2 = psp.tile([P, P], F32, space="PSUM", tag="mm")
                    nc.tensor.matmul(
                        out=pp2[:],
                        lhsT=hT[:],
                        rhs=W_sb[l + 1][:],
                        start=True,
                        stop=True,
                    )
                    hs2 = hs_keep[(l + 1) % 2][:, s * P : (s + 1) * P]
                    nc.scalar.activation(
                        out=hs2,
                        in_=pp2[:],
                        func=mybir.ActivationFunctionType.Copy,
                        scale=dinv_sb[:, s : s + 1],
                    )
                    nc.sync.dma_start(
                        out=cc_in[s * P : (s + 1) * P, :], in_=hs2
                    )
                    if (s + 1) % SPB == 0:
                        allgather_block(l + 1, s // SPB)
                else:
                    # MLP head: out = relu(h3@fc1a + qgT) @ fc2 + fc2_b
                    pm = psp.tile([P, P], F32, space="PSUM", tag="mm")
                    nc.tensor.matmul(
                        out=pm[:], lhsT=hT[:], rhs=fc1a_sb[:], start=True, stop=True
                    )
                    u = epi.tile([P, P], F32, tag="u")
                    nc.vector.tensor_tensor(
                        out=u[:],
                        in0=pm[:],
                        in1=qgT_sb[:, s * P : (s + 1) * P],
                        op=mybir.AluOpType.add,
                    )
                    ur = epi.tile([P, P], F32, tag="ur")
                    nc.scalar.activation(
                        out=ur[:], in_=u[:], func=mybir.ActivationFunctionType.Relu
                    )
                    pt2 = pst.tile([P, P], F32, space="PSUM", tag="pt")
                    nc.tensor.transpose(out=pt2[:], in_=ur[:], identity=ident[:])
                    uT = epi.tile([P, P], F32, tag="uT")
                    nc.scalar.copy(out=uT[:], in_=pt2[:])
                    po = psp.tile([P, OUTC], F32, space="PSUM", tag="mm")
                    nc.tensor.matmul(
                        out=po[:], lhsT=uT[:], rhs=fc2w_sb[:], start=True, stop=True
                    )
                    ob = epi.tile([P, OUTC], F32, tag="ob")
                    nc.vector.tensor_tensor(
                        out=ob[:], in0=po[:], in1=fc2bb_sb[:], op=mybir.AluOpType.add
                    )
                    nc.sync.dma_start(
                        out=out_d[s * P : (s + 1) * P, :], in_=ob[:]
                    )
    nc.compile()
    return nc


# ---------------------------------------------------------------- interface
_CACHE = {}


def kernel(**inputs):
    trace = bool(int(os.environ.get("GCN_TRACE", "0")))
    if trace:
        _install_axon_prof()
    import ml_dtypes
    from concourse.bass_utils import run_bass_kernel_spmd

    bf16 = ml_dtypes.bfloat16
    x = np.ascontiguousarray(np.asarray(inputs["x"], dtype=np.float32))
    qe = np.asarray(inputs["question_embedding"], dtype=np.float32)
    pp = preprocess(inputs["edge_index"], inputs["batch"])

    key = (
        tuple(pp["chunks_lo"].tolist()),
        tuple(pp["chunks_hi"].tolist()),
        pp["piece0_bound"],
    )
    if key not in _CACHE:
        _CACHE[key] = build_program(pp)
    nc = _CACHE[key]

    W = [np.asarray(inputs[f"W{i}"], np.float32) for i in range(3)]
    b = [np.asarray(inputs[f"b{i}"], np.float32) for i in range(3)]
    fc0_w = np.asarray(inputs["fc0_w"], np.float32)
    fc0_b = np.asarray(inputs["fc0_b"], np.float32)
    fc1_w = np.asarray(inputs["fc1_w"], np.float32)
    fc1_b = np.asarray(inputs["fc1_b"], np.float32)
    fc2_w = np.asarray(inputs["fc2_w"], np.float32)
    fc2_b = np.asarray(inputs["fc2_b"], np.float32)

    # host question path: qq = relu(qe@fc0 + fc0_b) @ fc1_w[128:] + fc1_b
    q = np.maximum(qe @ fc0_w + fc0_b, 0.0)
    qq = q @ fc1_w[P:] + fc1_b  # [G, 128]

    iota = np.broadcast_to(np.arange(P, dtype=np.float32), (P, P)).astype(bf16)
    common = {
        "iota_in": np.ascontiguousarray(iota),
        "W0": W[0].astype(bf16),
        "W1": W[1],
        "W2": W[2],
        "bb0": np.broadcast_to(b[0], (P, P)).copy(),
        "bb1": np.broadcast_to(b[1], (P, P)).copy(),
        "bb2": np.broadcast_to(b[2], (P, P)).copy(),
        "fc1a": np.ascontiguousarray(fc1_w[:P]),
        "fc2w": fc2_w,
        "fc2bb": np.broadcast_to(fc2_b, (P, OUTC)).copy(),
    }

    in_maps = []
    for c in range(NCORES):
        perm = pp["node_perm"][c]
        valid = perm >= 0
        xTc = np.zeros((P, SLOT_ROWS), dtype=np.float32)
        xTc[:, valid] = x[perm[valid]].T
        # qgT[p, s*128+f] = qq[batch[node(c,s,p)], f]
        gids = pp["gid_slot"][c]  # [P, TPC]
        qgTc = qq[gids].reshape(P, TPC * P).astype(np.float32)
        m = dict(common)
        m["xT"] = xTc.astype(bf16)
        m["idxW"] = np.ascontiguousarray(pp["idxW"][c])
        m["dstin"] = np.ascontiguousarray(pp["dstin_T"][c].astype(bf16))
        m["dinv_in"] = np.ascontiguousarray(pp["dinv_slot"][c])
        m["qgT"] = np.ascontiguousarray(qgTc)
        in_maps.append(m)

    res = run_bass_kernel_spmd(
        nc,
        in_maps,
        list(range(NCORES)),
        trace=trace,
    )
    kernel.last_result = res

    out = np.zeros((N, OUTC), dtype=np.float32)
    for c in range(NCORES):
        perm = pp["node_perm"][c]
        valid = perm >= 0
        out[perm[valid]] = res.results[c]["out"][valid]
    return out


# revision 31
# speedup vs baseline: 1.0220x; 1.0001x over previous
"""Trainium2 Bass kernel for the GCN model (nn_GCNModel_57853209477141).

Model: 3x GCNConv(128->128, sym-norm with self loops) with ReLU, question
embedding MLP, concat, 2-layer MLP head -> [50000, 32].

v2 design (8 NeuronCores, single SPMD launch):
- dst-node sharding: tiles of 128 nodes snake-dealt to cores by edge count;
  one compile-time schedule serves all 8 cores (SPMD).
- GCN norm factorization: agg[v] = dinv[v] * sum_e (dinv*h)[src_e]; tables
  store h~ = dinv*h in bf16, per-edge norm disappears.
- gather primitive: gpsimd.dma_gather (InstDMAGatherAnt), <=8 chunks
  (1024 rows) per instruction, int16 indices wrapped over 16 partitions.
  Table split at row 32768 into lo/hi halves to fit int16 range; each
  slot's edge stream is [lo chunks..., hi chunks...].
- aggregation per chunk: 0/1 one-hot [edge,dst] built on DVE via
  iota-compare, matmul-accumulated (bf16) into PSUM; self-loop term added
  from the resident h~ slice via identity matmul.
- single block-major table layout for all 3 layers; AllGather split into
  7 row-blocks fired as production completes (incl. layer 0) so collective
  traffic overlaps compute/gather.
- question path computed on host (0.1% of FLOPs), expanded per node with
  fc1 bias folded, loaded as a constant.

Host preprocessing: index/layout work + the tiny question MLP; all O(E*F)
and O(N*F*F) float work runs on device.
"""
import os
import sys
import types
from contextlib import ExitStack

import numpy as np

# ---------------------------------------------------------------- constants
N = 50000
E = 800000
G = 64
P = 128
NCORES = 8
TPC = 49  # tile slots per core
SPB = 7   # slots per AllGather block
NBLK = TPC // SPB  # 7
SLOT_ROWS = TPC * P  # 6272
NT = NCORES * SLOT_ROWS  # 50176
QD = 768
OUTC = 32
HALF = 32768  # int16 index split point
MAXC = 8  # max chunks (1024 rows) per dma_gather piece


def _install_axon_prof():
    """Register NTFF profile hook if the image's antenv lacks it; neuter
    bucket upload (zero-egress). Harmless when running without tracing."""
    try:
        from antenv import axon_hooks  # noqa: F401
    except ImportError:
        try:
            import antenv
            from trn_agent_boot.trn_boot import _ntff_profile_via_ctypes

            hook = _ntff_profile_via_ctypes("/opt/axon/libaxon_pjrt.so")
            mod = types.ModuleType("antenv.axon_hooks")
            mod.get_axon_ntff_profile_hook = lambda: hook
            mod.set_axon_ntff_profile_hook = lambda h: None
            sys.modules["antenv.axon_hooks"] = mod
            antenv.axon_hooks = mod
        except Exception:
            pass
    try:
        import concourse.bass_utils as bu

        bu.upload_artifacts = lambda tmpdir: "local://" + str(tmpdir)
    except Exception:
        pass


def _wrap_idxs16(lin: np.ndarray) -> np.ndarray:
    """[n] int -> [128, n//16] int16: wrapped col-major over 16 partitions
    (element i -> [i%16, i//16]), replicated 8x across partition groups."""
    n = lin.shape[0]
    assert n % 16 == 0
    w = np.asarray(lin, dtype=np.int16).reshape(n // 16, 16).T
    return np.tile(w, (8, 1))


# ---------------------------------------------------------------- host prep
def preprocess(edge_index, batch):
    src = np.asarray(edge_index[0], dtype=np.int64)
    dst = np.asarray(edge_index[1], dtype=np.int64)
    deg = (np.bincount(dst, minlength=N) + 1).astype(np.float64)
    dinv = np.where(deg > 0, 1.0 / np.sqrt(deg), 0.0).astype(np.float32)

    n_tiles = (N + P - 1) // P  # 391
    tile_of_node = np.arange(N) // P
    dst_tile = dst // P
    tile_counts = np.bincount(dst_tile, minlength=n_tiles)

    # snake-deal tiles (sorted by edge count desc) across cores
    order_all = np.argsort(-tile_counts, kind="stable")
    core_tiles = [[] for _ in range(NCORES)]
    for r in range(TPC):
        batch_t = order_all[r * NCORES : (r + 1) * NCORES]
        seq = range(NCORES) if r % 2 == 0 else range(NCORES - 1, -1, -1)
        for j, c in enumerate(seq):
            core_tiles[c].append(int(batch_t[j]) if j < len(batch_t) else -1)
    core_of_tile = np.full(n_tiles, -1, dtype=np.int64)
    slot_of_tile = np.full(n_tiles, -1, dtype=np.int64)
    for c in range(NCORES):
        for s, t in enumerate(core_tiles[c]):
            if t >= 0:
                core_of_tile[t] = c
                slot_of_tile[t] = s

    # block-major table row: [block][core][slot%SPB][pos]
    blk = slot_of_tile[tile_of_node] // SPB
    table_row = (
        blk * (NCORES * SPB * P)
        + core_of_tile[tile_of_node] * (SPB * P)
        + (slot_of_tile[tile_of_node] % SPB) * P
        + (np.arange(N) % P)
    )

    order = np.argsort(dst_tile, kind="stable")
    src_sorted = src[order]
    dst_sorted = dst[order]
    sorted_tiles = dst_tile[order]
    tile_starts = np.searchsorted(sorted_tiles, np.arange(n_tiles))
    tile_ends = np.searchsorted(sorted_tiles, np.arange(n_tiles), side="right")

    # per (core, slot): lo/hi edge streams (by table_row of src)
    lo_idx = [[None] * TPC for _ in range(NCORES)]
    hi_idx = [[None] * TPC for _ in range(NCORES)]
    lo_dst = [[None] * TPC for _ in range(NCORES)]
    hi_dst = [[None] * TPC for _ in range(NCORES)]
    for c in range(NCORES):
        for s in range(TPC):
            t = core_tiles[c][s]
            if t < 0:
                lo_idx[c][s] = np.zeros(0, np.int64)
                hi_idx[c][s] = np.zeros(0, np.int64)
                lo_dst[c][s] = np.zeros(0, np.int64)
                hi_dst[c][s] = np.zeros(0, np.int64)
                continue
            a, b = tile_starts[t], tile_ends[t]
            rows = table_row[src_sorted[a:b]]
            din = dst_sorted[a:b] % P
            m = rows < HALF
            # sort lo edges by table row: early gather pieces then only
            # depend on the first AllGather blocks (tighter base bound)
            rl, dl = rows[m], din[m]
            o = np.argsort(rl, kind="stable")
            lo_idx[c][s] = rl[o]
            lo_dst[c][s] = dl[o]
            hi_idx[c][s] = rows[~m] - HALF
            hi_dst[c][s] = din[~m]

    chunks_lo = np.zeros(TPC, dtype=np.int64)
    chunks_hi = np.zeros(TPC, dtype=np.int64)
    for s in range(TPC):
        for c in range(NCORES):
            chunks_lo[s] = max(chunks_lo[s], (len(lo_idx[c][s]) + P - 1) // P)
            chunks_hi[s] = max(chunks_hi[s], (len(hi_idx[c][s]) + P - 1) // P)
    nch = chunks_lo + chunks_hi
    TCH = int(nch.sum())
    TCHL = int(chunks_lo.sum())
    TCHH = int(chunks_hi.sum())
    lo_base = np.cumsum(np.concatenate([[0], chunks_lo[:-1]])).astype(int)
    hi_base = np.cumsum(np.concatenate([[0], chunks_hi[:-1]])).astype(int)

    # per-layer lo/hi chunk streams (slot-major); gather pieces of MAXC
    # chunks cut across slot boundaries so nearly all pieces are full.
    # dstin columns: lo stream at [0, TCHL), hi stream at [TCHL, TCH).
    dstin_T = np.full((NCORES, P, TCH), -1.0, dtype=np.float32)
    idxW = np.zeros((NCORES, P, 8 * TCH), dtype=np.int16)
    piece0_max = []
    for c in range(NCORES):
        lo_lin = np.zeros(TCHL * P, dtype=np.int64)
        lo_dl = np.full(TCHL * P, -1.0, dtype=np.float32)
        hi_lin = np.zeros(TCHH * P, dtype=np.int64)
        hi_dl = np.full(TCHH * P, -1.0, dtype=np.float32)
        for s in range(TPC):
            o = lo_base[s] * P
            nl = len(lo_idx[c][s])
            lo_lin[o : o + nl] = lo_idx[c][s]
            lo_dl[o : o + nl] = lo_dst[c][s]
            o = hi_base[s] * P
            nh = len(hi_idx[c][s])
            hi_lin[o : o + nh] = hi_idx[c][s]
            hi_dl[o : o + nh] = hi_dst[c][s]
        dstin_T[c, :, :TCHL] = lo_dl.reshape(TCHL, P).T
        dstin_T[c, :, TCHL:] = hi_dl.reshape(TCHH, P).T
        if TCHL:
            idxW[c, :, : 8 * TCHL] = _wrap_idxs16(lo_lin)
        if TCHH:
            idxW[c, :, 8 * TCHL :] = _wrap_idxs16(hi_lin)
        piece0_max.append(int(lo_lin[: MAXC * P].max()) if TCHL else 0)

    dinv_slot = np.zeros((NCORES, P, TPC), dtype=np.float32)
    gid_slot = np.zeros((NCORES, P, TPC), dtype=np.int64)
    node_perm = np.full((NCORES, SLOT_ROWS), -1, dtype=np.int64)
    batch = np.asarray(batch, dtype=np.int64)
    for c in range(NCORES):
        for s in range(TPC):
            t = core_tiles[c][s]
            if t < 0:
                continue
            v0 = t * P
            v1 = min(v0 + P, N)
            n = v1 - v0
            dinv_slot[c, :n, s] = dinv[v0:v1]
            gid_slot[c, :n, s] = batch[v0:v1]
            node_perm[c, s * P : s * P + n] = np.arange(v0, v1)

    # piece-0 base bound, quantized up to AG-block rows (block = 7168 rows)
    blk_rows = NCORES * SPB * P
    b0 = max(piece0_max) + 1 if piece0_max else HALF
    piece0_bound = min(-(-b0 // blk_rows) * blk_rows, NT)

    return dict(
        chunks_lo=chunks_lo,
        chunks_hi=chunks_hi,
        piece0_bound=piece0_bound,
        TCH=TCH,
        TCHL=TCHL,
        TCHH=TCHH,
        lo_base=lo_base,
        hi_base=hi_base,
        dstin_T=dstin_T,
        idxW=idxW,
        dinv_slot=dinv_slot,
        gid_slot=gid_slot,
        node_perm=node_perm,
    )


# ------------------------------------------------------------- bass program
def build_program(schedule):
    import concourse.bacc as bacc
    import concourse.bass as bass
    import concourse.tile as tile
    from concourse import mybir
    from concourse.masks import make_identity

    F32 = mybir.dt.float32
    BF16 = mybir.dt.bfloat16
    I16 = mybir.dt.int16
    chunks_lo = schedule["chunks_lo"]
    chunks_hi = schedule["chunks_hi"]
    piece0_bound = schedule["piece0_bound"]
    TCH = schedule["TCH"]
    TCHL = schedule["TCHL"]
    TCHH = schedule["TCHH"]
    lo_base = schedule["lo_base"]
    hi_base = schedule["hi_base"]

    nc = bacc.Bacc(
        "TRN2", target_bir_lowering=False, dynamic_dma_scratch_size=32768
    )
    dp = nc.declare_dram_parameter
    xT = dp("xT", [P, SLOT_ROWS], BF16, isOutput=False)
    idxW_in = dp("idxW", [P, 8 * TCH], I16, isOutput=False)
    dstin = dp("dstin", [P, TCH], BF16, isOutput=False)
    iota_in = dp("iota_in", [P, P], BF16, isOutput=False)
    dinv_in = dp("dinv_in", [P, TPC], F32, isOutput=False)
    W0_in = dp("W0", [P, P], BF16, isOutput=False)
    W1_in = dp("W1", [P, P], F32, isOutput=False)
    W2_in = dp("W2", [P, P], F32, isOutput=False)
    bb_in = [dp(f"bb{i}", [P, P], F32, isOutput=False) for i in range(3)]
    fc1a_in = dp("fc1a", [P, P], F32, isOutput=False)
    fc2w_in = dp("fc2w", [P, OUTC], F32, isOutput=False)
    fc2bb_in = dp("fc2bb", [P, OUTC], F32, isOutput=False)
    qgT_in = dp("qgT", [P, SLOT_ROWS], F32, isOutput=False)
    out_d = dp("out", [SLOT_ROWS, OUTC], F32, isOutput=True)

    cc_in = nc.dram_tensor("cc_in", [SLOT_ROWS, P], BF16)
    tables = [
        nc.dram_tensor(f"table{l}", [NT, P], BF16, addr_space="Shared")
        for l in range(3)
    ]

    # stream descriptors: (dstin col offset, idx col offset, total chunks)
    streams = [(0, 0, TCHL), (TCHL, 8 * TCHL, TCHH)]

    with tile.TileContext(nc) as tc, ExitStack() as ctx:
        const = ctx.enter_context(tc.tile_pool(name="const", bufs=1))
        gp = ctx.enter_context(tc.tile_pool(name="gp", bufs=16))
        ohp = ctx.enter_context(tc.tile_pool(name="ohp", bufs=8))
        psagg = ctx.enter_context(tc.tile_pool(name="psagg", bufs=3, space="PSUM"))
        psp = ctx.enter_context(tc.tile_pool(name="psp", bufs=3, space="PSUM"))
        pst = ctx.enter_context(tc.tile_pool(name="pst", bufs=2, space="PSUM"))
        epi = ctx.enter_context(tc.tile_pool(name="epi", bufs=3))

        # ---- constants (W0/dinv then xT in AG-block slices: production of
        # block 0 starts after ~230 KB instead of the whole 1.6 MB load)
        W0_sb = const.tile([P, P], BF16)
        nc.sync.dma_start(out=W0_sb[:], in_=W0_in[:])
        dinv_sb = const.tile([P, TPC], F32)
        nc.sync.dma_start(out=dinv_sb[:], in_=dinv_in[:])
        xT_sb = const.tile([P, SLOT_ROWS], BF16)
        for j in range(NBLK):
            c0, c1 = j * SPB * P, (j + 1) * SPB * P
            nc.sync.dma_start(out=xT_sb[:, c0:c1], in_=xT[:, c0:c1])

        # resident own-slice h~ buffers (self-loop source), layer parity
        hs_keep = [
            const.tile([P, SLOT_ROWS], BF16, tag=f"hsk{i}", name=f"hsk{i}")
            for i in range(2)
        ]

        def allgather_block(l, j):
            r0 = j * SPB * P
            r1 = (j + 1) * SPB * P
            nc.gpsimd.collective_compute(
                "AllGather",
                mybir.AluOpType.bypass,
                replica_groups=[list(range(NCORES))],
                ins=[cc_in[r0:r1].opt()],
                outs=[
                    tables[l][
                        j * NCORES * SPB * P : (j + 1) * NCORES * SPB * P
                    ].opt()
                ],
            )

        # ---- layer 0 production: h~0 = dinv * (x @ W0), AG fired per block
        for s in range(TPC):
            pp = psp.tile([P, P], F32, space="PSUM", tag="mm")
            nc.tensor.matmul(
                out=pp[:],
                lhsT=xT_sb[:, s * P : (s + 1) * P],
                rhs=W0_sb[:],
                start=True,
                stop=True,
            )
            hs = hs_keep[0][:, s * P : (s + 1) * P]
            nc.scalar.activation(
                out=hs,
                in_=pp[:],
                func=mybir.ActivationFunctionType.Copy,
                scale=dinv_sb[:, s : s + 1],
            )
            nc.sync.dma_start(out=cc_in[s * P : (s + 1) * P, :], in_=hs)
            if (s + 1) % SPB == 0:
                allgather_block(0, s // SPB)

        # ---- remaining constants: emitted AFTER production so their DMAs
        # don't occupy the sync/scalar queues ahead of production's
        # activations and cc_in writes (first consumer is at first gather)
        idxW_sb = const.tile([P, 8 * TCH], I16)
        nc.scalar.dma_start(out=idxW_sb[:], in_=idxW_in[:])
        dstin_sb = const.tile([P, TCH], BF16)
        nc.scalar.dma_start(out=dstin_sb[:], in_=dstin[:])
        iota_sb = const.tile([P, P], BF16)
        nc.sync.dma_start(out=iota_sb[:], in_=iota_in[:])
        W_sb = [None] * 3
        for i, win in ((1, W1_in), (2, W2_in)):
            w = const.tile([P, P], F32, tag=f"W{i}")
            nc.sync.dma_start(out=w[:], in_=win[:])
            W_sb[i] = w
        bb_sb = []
        for i in range(3):
            b = const.tile([P, P], F32, tag=f"bb{i}")
            nc.sync.dma_start(out=b[:], in_=bb_in[i][:])
            bb_sb.append(b)
        fc1a_sb = const.tile([P, P], F32)
        nc.sync.dma_start(out=fc1a_sb[:], in_=fc1a_in[:])
        fc2w_sb = const.tile([P, OUTC], F32)
        nc.sync.dma_start(out=fc2w_sb[:], in_=fc2w_in[:])
        fc2bb_sb = const.tile([P, OUTC], F32)
        nc.sync.dma_start(out=fc2bb_sb[:], in_=fc2bb_in[:])
        qgT_sb = const.tile([P, SLOT_ROWS], F32)
        nc.scalar.dma_start(out=qgT_sb[:], in_=qgT_in[:])
        ident = const.tile([P, P], F32)
        make_identity(nc, ident[:])
        ident_r = const.tile([P, P], BF16, tag="ident_r")
        nc.vector.tensor_copy(out=ident_r[:], in_=ident[:])

        # lo indices are < HALF, i.e. within AG blocks 0-4; bounding the
        # base AP there lets lo gathers start before AG blocks 5-6 land.
        LO_BOUND = 5 * NCORES * SPB * P  # 35840 >= HALF

        # ---- 3 aggregation layers
        for l in range(3):
            table = tables[l]
            piece_tiles = [{}, {}]  # per stream: piece id -> gather tile
            n_lo_pieces = (TCHL + MAXC - 1) // MAXC
            n_hi_pieces = (TCHH + MAXC - 1) // MAXC
            lo_issued = [0]
            hi_issued = [0]

            def get_piece(h, pj):
                if pj in piece_tiles[h]:
                    return piece_tiles[h][pj]
                _, coloff, tot = streams[h]
                pc = min(MAXC, tot - MAXC * pj)
                g = gp.tile([P, MAXC * P], BF16, tag="g")
                if h == 0:
                    lb = piece0_bound if pj == 0 else LO_BOUND
                    base = table[:lb, :]
                else:
                    base = table[HALF:, :]
                colbase = coloff + 8 * MAXC * pj
                nc.gpsimd.dma_gather(
                    g[:, : pc * P].rearrange("p (k c) -> p k c", k=pc),
                    base,
                    idxW_sb[:, colbase : colbase + pc * 8],
                    pc * P,
                    pc * P,
                    P,
                )
                piece_tiles[h][pj] = g
                return g

            def prefetch_lo(upto):
                while lo_issued[0] < min(upto, n_lo_pieces):
                    get_piece(0, lo_issued[0])
                    lo_issued[0] += 1

            def prefetch_hi(upto):
                while hi_issued[0] < min(upto, n_hi_pieces):
                    get_piece(1, hi_issued[0])
                    hi_issued[0] += 1

            for s in range(TPC):
                prefetch_lo(int(lo_base[s]) // MAXC + 10)
                prefetch_hi(int(hi_base[s]) // MAXC + 3)
                ps = psagg.tile([P, P], F32, space="PSUM", tag="agg")
                first = True
                for h, b0, cnt in (
                    (0, int(lo_base[s]), int(chunks_lo[s])),
                    (1, int(hi_base[s]), int(chunks_hi[s])),
                ):
                    doff = streams[h][0]
                    for k in range(cnt):
                        kk = b0 + k
                        g = get_piece(h, kk // MAXC)
                        kp = kk % MAXC
                        oh = ohp.tile([P, P], BF16, tag="oh")
                        nc.vector.tensor_tensor(
                            out=oh[:],
                            in0=dstin_sb[
                                :, doff + kk : doff + kk + 1
                            ].to_broadcast([P, P]),
                            in1=iota_sb[:],
                            op=mybir.AluOpType.is_equal,
                        )
                        nc.tensor.matmul(
                            out=ps[:],
                            lhsT=oh[:],
                            rhs=g[:, kp * P : (kp + 1) * P],
                            start=first,
                            stop=False,
                        )
                        first = False
                # self-loop term
                nc.tensor.matmul(
                    out=ps[:],
                    lhsT=ident_r[:],
                    rhs=hs_keep[l % 2][:, s * P : (s + 1) * P],
                    start=first,
                    stop=True,
                )
                # epilogue: h = relu(dinv*agg + b)
                t1 = epi.tile([P, P], F32, tag="t1")
                nc.scalar.activation(
                    out=t1[:],
                    in_=ps[:],
                    func=mybir.ActivationFunctionType.Copy,
                    scale=dinv_sb[:, s : s + 1],
                )
                t2 = epi.tile([P, P], F32, tag="t2")
                nc.vector.tensor_tensor(
                    out=t2[:], in0=t1[:], in1=bb_sb[l][:], op=mybir.AluOpType.add
                )
                hrelu = epi.tile([P, P], F32, tag="hrelu")
                nc.scalar.activation(
                    out=hrelu[:],
                    in_=t2[:],
                    func=mybir.ActivationFunctionType.Relu,
                )
                pt = pst.tile([P, P], F32, space="PSUM", tag="pt")
                nc.tensor.transpose(out=pt[:], in_=hrelu[:], identity=ident[:])
                hT = epi.tile([P, P], F32, tag="hT")
                nc.scalar.copy(out=hT[:], in_=pt[:])
                if l < 2:
                    pp2 = psp.tile([P, P], F32, space="PSUM", tag="mm")
                    nc.tensor.matmul(
                        out=pp2[:],
                        lhsT=hT[:],
                        rhs=W_sb[l + 1][:],
                        start=True,
                        stop=True,
                    )
                    hs2 = hs_keep[(l + 1) % 2][:, s * P : (s + 1) * P]
                    nc.scalar.activation(
                        out=hs2,
                        in_=pp2[:],
                        func=mybir.ActivationFunctionType.Copy,
                        scale=dinv_sb[:, s : s + 1],
                    )
                    nc.sync.dma_start(
                        out=cc_in[s * P : (s + 1) * P, :], in_=hs2
                    )
                    if (s + 1) % SPB == 0:
                        allgather_block(l + 1, s // SPB)
                else:
                    # MLP head: out = relu(h3@fc1a + qgT) @ fc2 + fc2_b
                    pm = psp.tile([P, P], F32, space="PSUM", tag="mm")
                    nc.tensor.matmul(
                        out=pm[:], lhsT=hT[:], rhs=fc1a_sb[:], start=True, stop=True
                    )
                    u = epi.tile([P, P], F32, tag="u")
                    nc.vector.tensor_tensor(
                        out=u[:],
                        in0=pm[:],
                        in1=qgT_sb[:, s * P : (s + 1) * P],
                        op=mybir.AluOpType.add,
                    )
                    ur = epi.tile([P, P], F32, tag="ur")
                    nc.scalar.activation(
                        out=ur[:], in_=u[:], func=mybir.ActivationFunctionType.Relu
                    )
                    pt2 = pst.tile([P, P], F32, space="PSUM", tag="pt")
                    nc.tensor.transpose(out=pt2[:], in_=ur[:], identity=ident[:])
                    uT = epi.tile([P, P], F32, tag="uT")
                    nc.scalar.copy(out=uT[:], in_=pt2[:])
                    po = psp.tile([P, OUTC], F32, space="PSUM", tag="mm")
                    nc.tensor.matmul(
                        out=po[:], lhsT=uT[:], rhs=fc2w_sb[:], start=True, stop=True
                    )
                    ob = epi.tile([P, OUTC], F32, tag="ob")
                    nc.vector.tensor_tensor(
                        out=ob[:], in0=po[:], in1=fc2bb_sb[:], op=mybir.AluOpType.add
                    )
                    nc.sync.dma_start(
                        out=out_d[s * P : (s + 1) * P, :], in_=ob[:]
                    )
    nc.compile()
    return nc


# ---------------------------------------------------------------- interface
_CACHE = {}


def kernel(**inputs):
    trace = bool(int(os.environ.get("GCN_TRACE", "0")))
    if trace:
        _install_axon_prof()
    import ml_dtypes
    from concourse.bass_utils import run_bass_kernel_spmd

    bf16 = ml_dtypes.bfloat16
    x = np.ascontiguousarray(np.asarray(inputs["x"], dtype=np.float32))
    qe = np.asarray(inputs["question_embedding"], dtype=np.float32)
    pp = preprocess(inputs["edge_index"], inputs["batch"])

    key = (
        tuple(pp["chunks_lo"].tolist()),
        tuple(pp["chunks_hi"].tolist()),
        pp["piece0_bound"],
    )
    if key not in _CACHE:
        _CACHE[key] = build_program(pp)
    nc = _CACHE[key]

    W = [np.asarray(inputs[f"W{i}"], np.float32) for i in range(3)]
    b = [np.asarray(inputs[f"b{i}"], np.float32) for i in range(3)]
    fc0_w = np.asarray(inputs["fc0_w"], np.float32)
    fc0_b = np.asarray(inputs["fc0_b"], np.float32)
    fc1_w = np.asarray(inputs["fc1_w"], np.float32)
    fc1_b = np.asarray(inputs["fc1_b"], np.float32)
    fc2_w = np.asarray(inputs["fc2_w"], np.float32)
    fc2_b = np.asarray(inputs["fc2_b"], np.float32)

    # host question path: qq = relu(qe@fc0 + fc0_b) @ fc1_w[128:] + fc1_b
    q = np.maximum(qe @ fc0_w + fc0_b, 0.0)
    qq = q @ fc1_w[P:] + fc1_b  # [G, 128]

    iota = np.broadcast_to(np.arange(P, dtype=np.float32), (P, P)).astype(bf16)
    common = {
        "iota_in": np.ascontiguousarray(iota),
        "W0": W[0].astype(bf16),
        "W1": W[1],
        "W2": W[2],
        "bb0": np.broadcast_to(b[0], (P, P)).copy(),
        "bb1": np.broadcast_to(b[1], (P, P)).copy(),
        "bb2": np.broadcast_to(b[2], (P, P)).copy(),
        "fc1a": np.ascontiguousarray(fc1_w[:P]),
        "fc2w": fc2_w,
        "fc2bb": np.broadcast_to(fc2_b, (P, OUTC)).copy(),
    }

    in_maps = []
    for c in range(NCORES):
        perm = pp["node_perm"][c]
        valid = perm >= 0
        xTc = np.zeros((P, SLOT_ROWS), dtype=np.float32)
        xTc[:, valid] = x[perm[valid]].T
        # qgT[p, s*128+f] = qq[batch[node(c,s,p)], f]
        gids = pp["gid_slot"][c]  # [P, TPC]
        qgTc = qq[gids].reshape(P, TPC * P).astype(np.float32)
        m = dict(common)
        m["xT"] = xTc.astype(bf16)
        m["idxW"] = np.ascontiguousarray(pp["idxW"][c])
        m["dstin"] = np.ascontiguousarray(pp["dstin_T"][c].astype(bf16))
        m["dinv_in"] = np.ascontiguousarray(pp["dinv_slot"][c])
        m["qgT"] = np.ascontiguousarray(qgTc)
        in_maps.append(m)

    res = run_bass_kernel_spmd(
        nc,
        in_maps,
        list(range(NCORES)),
        trace=trace,
    )
    kernel.last_result = res

    out = np.zeros((N, OUTC), dtype=np.float32)
    for c in range(NCORES):
        perm = pp["node_perm"][c]
        valid = perm >= 0
        out[perm[valid]] = res.results[c]["out"][valid]
    return out
